# revision 28
# baseline (speedup 1.0000x reference)
"""DMRG two-site effective Hamiltonian application (ApplyMPO) on 8 trn2 cores.

Math (reference):
  res[h,i,j,k] = sum_{a,b,c,d,e,f,g} L[b,h,a] M1[b,d,i,c] M2[d,f,j,e]
                                     R[f,k,g] psi[a,c,e,g]

Device algorithm (per core, output bond h sharded 8 x 128), all bf16 with
fp32 PSUM accumulation:
  Q[(b,c,e),(i,j,f)] = sum_d M1[b,d,i,c] M2[d,f,j,e]            (host, 400 els)
  step1: T1[h; pack, (g6,bce)] = sum_a L[b,h,a] psi[a,(c,e),g]  (PE, K=a)
         written in 128-col packs: col = g6*20 + (b*4+ce), 6 g's per pack
  flipA: T1P[(g6,bce); pack, h] = DMA XBAR transpose of T1       (DMA engine)
  mix:   T3H[h; (i,j,f), g]     = T1P[pack]^T @ Q6P              (PE, K=20 eff)
  flipB: T3G[g; (ijf,blk), h]   = one DMA XBAR transpose of T3H  (q<3; q3: PE)
  step4: res[h; ij,k]          += T3G[ijf,blk]^T @ R^T[f][g,k]   (PE, K=g, acc f)

Scheduling notes (from trace analysis):
- PE is the bottleneck (~85% busy).  flipB rides the XBAR for q<3 (one
  DmaTransposeAnt per quarter: each 128-col chunk of flat T3H is exactly one
  [h; g128] tile); q3 keeps flipB on the PE so its step-4 groups can
  interleave into the flipB loop at the tail.
- flipA stays split across the two HWDGE rings as in the tuned baseline;
  flipB(q) goes on the sync ring after the flipA halves.  Consumers of
  t3g(q) fire no earlier than b4 of step1(q+1), ~14us after mix(q), by
  which time the flipB transfer (~6.5us) has landed.
- A 10-matmul warmup accumulation group (no intermediate semaphores, so it
  runs back-to-back) on a memset tile keeps the PE HAM activity window busy
  from ~5us so the clock gate reaches 2.4 GHz before the real matmuls;
  otherwise the first ~18us run at 1.2 GHz.
- DMA trigger instructions cost ~600ns of sequencer time each, so psi/rt
  loads are batched (one trigger per quarter; q0's psi is split so the very
  first matmul gates on a 128KB slice, with the tail slice on the idle
  SWDGE ring).
- res accumulates fp32 in SBUF; the final (q3) add writes a bf16 staging
  tile DMAed out per 512-col chunk on the scalar ring (idle at the tail).
  The last group is split into two 256-col chunks to shorten the
  post-last-matmul tail.
"""

import numpy as np
import ml_dtypes

import concourse.bacc as bacc
import concourse.mybir as mybir
import concourse.tile as tile
from concourse import bass_utils

F32 = mybir.dt.float32
BF16 = mybir.dt.bfloat16
BF_NP = ml_dtypes.bfloat16

CHI = 1024
W = 5
D = 2
NCORES = 8
H = CHI // NCORES  # 128, h rows per core
NPACK = 43  # 6-g packs per 256-g quarter: 42 full + one 4-g tail

_nc_cache = None


def _build_nc():
    nc = bacc.Bacc("TRN2", target_bir_lowering=False)
    # host-prearranged: psi[ac, q, a_lo, ce, g256]; lt[b, a_lo, ac, h]; rt[blk, g_lo, f, k]
    psi = nc.dram_tensor("psi", [8, 4, 128, 4, 256], BF16, kind="ExternalInput")
    lt = nc.dram_tensor("lt", [5, 128, 8, H], BF16, kind="ExternalInput")
    rt = nc.dram_tensor("rt", [8, 128, 5, 1024], BF16, kind="ExternalInput")
    q6 = nc.dram_tensor("q6", [120, 128], BF16, kind="ExternalInput")
    q4 = nc.dram_tensor("q4", [80, 128], BF16, kind="ExternalInput")
    idn = nc.dram_tensor("idn", [128, 128], BF16, kind="ExternalInput")
    res = nc.dram_tensor("res", [H, 4096], BF16, kind="ExternalOutput")  # h;(i,j,k)

    with tile.TileContext(nc) as tc:
        with (
            tc.tile_pool(name="const", bufs=1) as const_pool,
            tc.tile_pool(name="psis", bufs=2) as psi_pool,
            tc.tile_pool(name="t1", bufs=2) as t1_pool,
            tc.tile_pool(name="t1p", bufs=2) as t1p_pool,
            tc.tile_pool(name="t3h", bufs=2) as t3h_pool,
            tc.tile_pool(name="t3g", bufs=2) as t3g_pool,
            tc.tile_pool(name="rt_p", bufs=2) as rt_pool,
            tc.tile_pool(name="resp", bufs=1) as res_pool,
            tc.tile_pool(name="ps_s1", bufs=2, space="PSUM") as ps_s1,
            tc.tile_pool(name="ps_mid", bufs=2, space="PSUM") as ps_mid,
            tc.tile_pool(name="ps_s4", bufs=2, space="PSUM") as ps_s4,
        ):
            # ---- PE warmup: one accumulation group (no inter-matmul
            # semaphores -> truly back-to-back) so the HAM clock gate opens
            # (1.2 -> 2.4 GHz) before the real matmuls start
            warm_src = const_pool.tile([128, 512], BF16)
            nc.vector.memset(warm_src[:], 1.0)
            wps = ps_s4.tile([128, 512], F32, tag="s4")
            NWARM = 10
            for i in range(NWARM):
                nc.tensor.matmul(
                    wps[:], warm_src[:, 0:128], warm_src[:],
                    start=(i == 0), stop=(i == NWARM - 1),
                )

            # ---- static loads (only b=0 weights gate the first matmul) ----
            lt_sb = const_pool.tile([128, 5, 8, H], BF16)  # [a_lo; b, ac, h]
            lt_r = lt.ap().rearrange("b p ac h -> p b ac h")
            nc.scalar.dma_start(lt_sb[:, 0, 0], lt_r[:, 0, 0])
            nc.scalar.dma_start(lt_sb[:, 0, 1:8], lt_r[:, 0, 1:8])
            q6_sb = const_pool.tile([120, 128], BF16)
            q4_sb = const_pool.tile([80, 128], BF16)
            idn_sb = const_pool.tile([128, 128], BF16)
            res_sb = res_pool.tile([128, 4096], F32)
            res_bf = res_pool.tile([128, 4096], BF16)

            def load_rest_of_consts():
                for b in range(1, 5):
                    nc.scalar.dma_start(lt_sb[:, b], lt_r[:, b])
                nc.scalar.dma_start(q6_sb[:], q6.ap())
                nc.scalar.dma_start(q4_sb[:], q4.ap())
                nc.scalar.dma_start(idn_sb[:], idn.ap())

            evac_ct = 0

            def evac_copy(out, in_):
                # DVE is ~2x faster than ACT for copies; give ACT 1 in 3.
                nonlocal evac_ct
                evac_ct += 1
                if evac_ct % 3 == 0:
                    nc.scalar.copy(out, in_)
                else:
                    nc.vector.tensor_copy(out, in_)

            # deferred step-4 work: list of closures (one per psum group)
            pending_s4 = []

            def fire_s4(n=1):
                for _ in range(n):
                    if pending_s4:
                        pending_s4.pop(0)()

            psi_r = psi.ap()

            def load_psi(qq, gate=None):
                # one SBUF tile per quarter: [a_lo; ac, ce, g256]
                pt = psi_pool.tile([128, 8, 4, 256], BF16, tag="psi")
                if gate is not None:
                    # tiny copy from the gate region into the DMA dest: the
                    # transfer then cannot start before the gate is written,
                    # so this prefetch does not steal HBM bandwidth from the
                    # load the PE is currently consuming (startup only)
                    nc.gpsimd.tensor_copy(pt[0:2, 0, 0, 0:2], gate)
                if qq == 0:
                    # split so the first matmuls gate on small slices; the
                    # ac6-7 tail rides the otherwise-idle SWDGE ring
                    nc.sync.dma_start(pt[:, 0, 0:2], psi_r[0, 0, :, 0:2])
                    nc.sync.dma_start(pt[:, 0, 2:4], psi_r[0, 0, :, 2:4])
                    nc.sync.dma_start(
                        pt[:, 1:3], psi_r[1:3, 0].rearrange("ac p ce g -> p ac ce g")
                    )
                    nc.sync.dma_start(
                        pt[:, 3:6], psi_r[3:6, 0].rearrange("ac p ce g -> p ac ce g")
                    )
                    nc.gpsimd.dma_start(
                        pt[:, 6:8], psi_r[6:8, 0].rearrange("ac p ce g -> p ac ce g")
                    )
                else:
                    # prefetches ride the SWDGE (Pool) ring: the sync ring
                    # must stay clear so flipB(q) dispatches right after
                    # flipA(q) instead of behind ~13us of prefetch transfers
                    nc.gpsimd.dma_start(
                        pt[:], psi_r[:, qq].rearrange("ac p ce g -> p ac ce g")
                    )
                return pt

            def load_rt(qq, gate=None):
                # both 128-g blocks of the quarter in one trigger
                rt_t = rt_pool.tile([128, 2, 5, 1024], BF16, tag="rt")
                if gate is not None:
                    nc.gpsimd.tensor_copy(rt_t[0:2, 0, 0, 0:2], gate)
                nc.gpsimd.dma_start(
                    rt_t[:],
                    rt.ap()[qq * 2 : qq * 2 + 2].rearrange("blk p f k -> p blk f k"),
                )
                return rt_t

            def alloc_t1q():
                t1q = t1_pool.tile([128, NPACK, 128], BF16, tag="t1q")
                # packed destination views: col = g6*20 + (b*4 + ce)
                dstA = t1q[:, 0:42, 0:120].rearrange("p n (g c) -> p n g c", c=20)
                dstB = t1q[:, 42, 0:80].rearrange("p (g c) -> p g c", c=20)
                return {"t1q": t1q, "dstA": dstA, "dstB": dstB}

            def step1_b(st, psi_t, b):
                ps1 = ps_s1.tile([128, 4, 256], F32, tag="s1")  # 2 banks
                ps1_flat = ps1[:].rearrange("p c g -> p (c g)")
                for ac in range(8):
                    lhsT = lt_sb[:, b, ac]
                    psi_flat = psi_t[:, ac].rearrange("p c g -> p (c g)")
                    for cep in range(2):  # one 512-wide MM per PSUM bank
                        nc.tensor.matmul(
                            ps1_flat[:, cep * 512 : (cep + 1) * 512],
                            lhsT,
                            psi_flat[:, cep * 512 : (cep + 1) * 512],
                            start=(ac == 0),
                            stop=(ac == 7),
                        )
                nc.vector.tensor_copy(
                    st["dstA"][:, :, :, b * 4 : b * 4 + 2],
                    ps1[:, 0:2, 0:252].rearrange("p c (n g) -> p n g c", g=6),
                )
                nc.scalar.copy(
                    st["dstA"][:, :, :, b * 4 + 2 : b * 4 + 4],
                    ps1[:, 2:4, 0:252].rearrange("p c (n g) -> p n g c", g=6),
                )
                nc.vector.tensor_copy(
                    st["dstB"][:, :, b * 4 : b * 4 + 2],
                    ps1[:, 0:2, 252:256].rearrange("p c g -> p g c"),
                )
                nc.scalar.copy(
                    st["dstB"][:, :, b * 4 + 2 : b * 4 + 4],
                    ps1[:, 2:4, 252:256].rearrange("p c g -> p g c"),
                )

            # ---------- step 4: defer per-(ij,kh) groups into q+1's body
            last_ps4 = [None]

            def make_s4(qq, t3g_, rt_t):
                def emit(ij, kh, k0, k1):
                    ps4 = ps_s4.tile([128, 512], F32, tag="s4")  # 1 bank
                    last_ps4[0] = ps4
                    kw = k1 - k0
                    for blk2 in range(2):
                        for f in range(5):
                            nc.tensor.matmul(
                                ps4[:, 0:kw],
                                t3g_[:, (ij * 5 + f) * 2 + blk2, :],
                                rt_t[:, blk2, f, kh * 512 + k0 : kh * 512 + k1],
                                start=(blk2 == 0 and f == 0),
                                stop=(blk2 == 1 and f == 4),
                            )
                    c0 = ij * 1024 + kh * 512 + k0
                    if qq == 0:
                        evac_copy(res_sb[:, c0 : c0 + kw], ps4[:, 0:kw])
                    elif qq < 3:
                        nc.vector.tensor_add(
                            res_sb[:, c0 : c0 + kw],
                            res_sb[:, c0 : c0 + kw],
                            ps4[:, 0:kw],
                        )
                    else:
                        # final quarter: add into the bf16 staging tile and
                        # ship it out on the scalar ring (idle at the tail)
                        nc.vector.tensor_add(
                            res_bf[:, c0 : c0 + kw],
                            res_sb[:, c0 : c0 + kw],
                            ps4[:, 0:kw],
                        )
                        nc.scalar.dma_start(
                            res.ap()[:, c0 : c0 + kw], res_bf[:, c0 : c0 + kw]
                        )

                groups = []
                for ij in range(4):
                    for kh in range(2):
                        if qq == 3 and ij == 3 and kh == 1:
                            # split the very last group so the final
                            # accumulate+DMA covers only 256 columns
                            groups.append(lambda ij=ij, kh=kh: emit(ij, kh, 0, 256))
                            groups.append(lambda ij=ij, kh=kh: emit(ij, kh, 256, 512))
                        else:
                            groups.append(lambda ij=ij, kh=kh: emit(ij, kh, 0, 512))
                return groups

            st_by_q = {}
            psis_by_q = {}
            rt_by_q = {}
            for q in range(4):  # g-quarters
                # ---------- step 1: T1q[h; pack, (g6, bce)] ----------
                if q == 0:
                    psis_by_q[0] = load_psi(0)
                    st_by_q[0] = alloc_t1q()
                    load_rest_of_consts()
                # at startup, gate the q1/rt0 prefetches on step1(q0)'s b2
                # output so psi(q0) gets the full HBM pipe first
                g0 = (
                    st_by_q[0]["t1q"][0:2, 0, 8:10] if q == 0 else None
                )
                if q < 3:
                    psis_by_q[q + 1] = load_psi(q + 1, gate=g0)
                rt_by_q[q] = load_rt(q, gate=g0)
                # leading b-blocks of this quarter were hoisted into the
                # previous body (3 blocks into q0's flipA window, else 2)
                for b in range(0 if q == 0 else (3 if q == 1 else 2), 5):
                    step1_b(st_by_q[q], psis_by_q[q], b)

                # ---------- flipA: DMA XBAR transpose to T1P[(g6,bce); pack, h]
                # ALL XBAR transposes stay on the sync ring: two transposes
                # in flight from different rings corrupt each other (the
                # framework's serialization guard does not cover the
                # cross-ring case -- observed as 15-of-every-16 h rows
                # garbage, the XBAR tile height).
                t1q_flat = st_by_q[q]["t1q"][:].rearrange("p n c -> p (n c)")
                t1p = t1p_pool.tile([128, NPACK, 128], BF16, tag="t1p")
                with tc.high_priority():
                    nc.sync.dma_start(t1p[:], t1q_flat[:], transpose=True)
                # hoist the next quarter's first two step-1 blocks here: they
                # keep the PE busy while the flipA XBAR transpose lands
                if q < 3:
                    st_by_q[q + 1] = alloc_t1q()
                    step1_b(st_by_q[q + 1], psis_by_q[q + 1], 0)
                    fire_s4(1)
                    step1_b(st_by_q[q + 1], psis_by_q[q + 1], 1)
                    fire_s4(1)
                    if q == 0:  # q0 has no deferred step-4 filler
                        step1_b(st_by_q[1], psis_by_q[1], 2)
                    fire_s4(1)
                else:
                    fire_s4(3)
                fire_s4(1)
                # Force mix(q) to sequence AFTER the window work in the PE
                # stream: the scheduler's cost model underestimates the flipA
                # transfer and otherwise places mix's first matmuls (which
                # wait on flipA) ahead of the ready fires -- head-of-line
                # blocking that idles the PE and re-throttles the HAM clock
                # gate.  A bypass-op rewrite of a sliver of every t1p pack
                # (out = in0 = the t1p data itself, in1 = the last fire's
                # PSUM as a pure dependency) makes every mix matmul order
                # after the cover.
                if q > 0 and last_ps4[0] is not None:
                    nc.vector.tensor_tensor(
                        t1p[0:2, :, 0:2],
                        t1p[0:2, :, 0:2],
                        last_ps4[0][0:2, 0:86].rearrange("p (n c) -> p n c", c=2),
                        op=mybir.AluOpType.bypass,
                    )

                # ---------- mix: T3H[h; ijf20, g256] ----------
                t3h = t3h_pool.tile([128, 20, 256], BF16, tag="t3h")
                groups = [list(range(g0, min(g0 + 4, NPACK))) for g0 in range(0, NPACK, 4)]
                for gi, grp in enumerate(groups):
                    pmq = ps_mid.tile([128, 4, 128], F32, tag="mid")  # 1 bank
                    for k, pack in enumerate(grp):
                        if pack < 42:
                            nc.tensor.matmul(
                                pmq[:, k, :], t1p[0:120, pack, :], q6_sb[:],
                                start=True, stop=True,
                            )
                        else:
                            nc.tensor.matmul(
                                pmq[:, k, :], t1p[0:80, 42, :], q4_sb[:],
                                start=True, stop=True,
                            )
                    nfull = sum(1 for p_ in grp if p_ < 42)
                    if nfull:
                        evac_copy(
                            t3h[:, :, grp[0] * 6 : grp[0] * 6 + nfull * 6].rearrange(
                                "p i (k g) -> p k i g", g=6
                            ),
                            pmq[:, 0:nfull, 0:120].rearrange(
                                "p k (i g) -> p k i g", g=6
                            ),
                        )
                    if grp[-1] == 42:
                        evac_copy(
                            t3h[:, :, 252:256],
                            pmq[:, nfull, 0:80].rearrange("p (i g) -> p i g", g=4),
                        )
                    if gi in (2, 5):
                        fire_s4(1)
                fire_s4(2)

                # ---------- flipB: T3G[g; (ijf, blk), h] ----------
                t3g = t3g_pool.tile([128, 40, 128], BF16, tag="t3g")
                if q < 3:
                    # one XBAR transpose: every 128-col chunk of flat T3H is
                    # exactly one [h; g128] tile for (ijf = n//2, blk = n%2)
                    t3h_flat = t3h[:].rearrange("p i g -> p (i g)")
                    with tc.high_priority():
                        nc.sync.dma_start(t3g[:], t3h_flat, transpose=True)
                    pending_s4.extend(make_s4(q, t3g, rt_by_q[q]))
                else:
                    # last quarter: keep flipB on the PE so its step-4 groups
                    # interleave in as soon as their t3g slices are ready
                    pending_s4.extend(make_s4(q, t3g, rt_by_q[q]))
                    for ng in range(10):  # 4 transposes per PSUM bank
                        pb = ps_mid.tile([128, 4, 128], BF16, tag="mid")
                        for j in range(4):
                            n = ng * 4 + j  # n = ijf*2 + blk2
                            nc.tensor.transpose(
                                pb[:, j, :],
                                t3h[:, n // 2, (n % 2) * 128 : (n % 2) * 128 + 128],
                                idn_sb[:],
                            )
                        evac_copy(
                            t3g[:, ng * 4 : (ng + 1) * 4, :].rearrange(
                                "p n h -> p (n h)"
                            ),
                            pb[:].rearrange("p j h -> p (j h)"),
                        )
                        if ng in (3, 6):
                            fire_s4(2)  # ij0 after n<=15 done, ij1 after n<=27

            # flush remaining deferred step-4 work (last quarter)
            fire_s4(len(pending_s4))
    nc.compile()
    return nc


def _host_inputs(psi_flat, L, M1, M2, R):
    # psi[a,ce,g] -> [ac, q, a_lo, ce, g256]
    psi = np.ascontiguousarray(
        psi_flat.reshape(8, 128, 4, 4, 256).transpose(0, 3, 1, 2, 4)
    ).astype(BF_NP)
    # R[f,k,g] -> RT[f,g,k] -> [blk, g_lo, f, k]
    RT = np.ascontiguousarray(
        R.transpose(2, 0, 1).reshape(8, 128, 5, 1024)
    ).astype(BF_NP)
    Q = np.einsum("bdic,dfje->bceijf", M1, M2).reshape(20, 20).astype(np.float32)
    rows = np.arange(20)
    Q6P = np.zeros((120, 128), np.float32)
    for g6 in range(6):
        Q6P[np.ix_(g6 * 20 + rows, rows * 6 + g6)] = Q
    Q4P = np.zeros((80, 128), np.float32)
    for g4 in range(4):
        Q4P[np.ix_(g4 * 20 + rows, rows * 4 + g4)] = Q
    Q6P = Q6P.astype(BF_NP)
    Q4P = Q4P.astype(BF_NP)
    idn = np.eye(128, dtype=np.float32).astype(BF_NP)
    in_maps = []
    for c in range(NCORES):
        LT = np.ascontiguousarray(
            L[:, c * H : (c + 1) * H, :].transpose(0, 2, 1).reshape(5, 8, 128, H)
            .transpose(0, 2, 1, 3)
        ).astype(BF_NP)  # [b, a_lo, ac, h]
        in_maps.append({"psi": psi, "lt": LT, "rt": RT, "q6": Q6P, "q4": Q4P, "idn": idn})
    return in_maps


def kernel(**inputs):
    psi_flat = np.asarray(inputs["psi_flat"], np.float32)
    L = np.asarray(inputs["L"], np.float32)
    M1 = np.asarray(inputs["M1"], np.float32)
    M2 = np.asarray(inputs["M2"], np.float32)
    R = np.asarray(inputs["R"], np.float32)

    global _nc_cache
    if _nc_cache is None:
        _nc_cache = _build_nc()
    nc = _nc_cache

    in_maps = _host_inputs(psi_flat, L, M1, M2, R)
    out = bass_utils.run_bass_kernel_spmd(nc, in_maps, core_ids=list(range(NCORES)))
    parts = [
        np.asarray(out.results[c]["res"]).astype(np.float32) for c in range(NCORES)
    ]
    return np.concatenate(parts, axis=0).reshape(-1)


# revision 32
# speedup vs baseline: 1.0846x; 1.0846x over previous
"""DMRG two-site effective Hamiltonian application (ApplyMPO) on 8 trn2 cores.

Math (reference):
  res[h,i,j,k] = sum_{a,b,c,d,e,f,g} L[b,h,a] M1[b,d,i,c] M2[d,f,j,e]
                                     R[f,k,g] psi[a,c,e,g]

Device algorithm (per core, output bond h sharded 8 x 128), all bf16 with
fp32 PSUM accumulation:
  Q[(b,c,e),(i,j,f)] = sum_d M1[b,d,i,c] M2[d,f,j,e]            (host, 400 els)
  step1: T1[h; pack, (g6,bce)] = sum_a L[b,h,a] psi[a,(c,e),g]  (PE, K=a)
         written in 128-col packs: col = g6*20 + (b*4+ce), 6 g's per pack
  flipA: T1P[(g6,bce); pack, h] = DMA XBAR transpose of T1       (DMA engine)
  mix:   T3H[h; (i,j,f), g]     = T1P[pack]^T @ Q6P              (PE, K=20 eff)
  flipB: T3G[g; (ijf,blk), h]   = one DMA XBAR transpose of T3H  (q<3; q3: PE)
  step4: res[h; ij,k]          += T3G[ijf,blk]^T @ R^T[f][g,k]   (PE, K=g, acc f)

Scheduling notes (from trace analysis):
- PE is the bottleneck (~85% busy).  flipB rides the XBAR for q<3 (one
  DmaTransposeAnt per quarter: each 128-col chunk of flat T3H is exactly one
  [h; g128] tile); q3 keeps flipB on the PE so its step-4 groups can
  interleave into the flipB loop at the tail.
- flipA stays split across the two HWDGE rings as in the tuned baseline;
  flipB(q) goes on the sync ring after the flipA halves.  Consumers of
  t3g(q) fire no earlier than b4 of step1(q+1), ~14us after mix(q), by
  which time the flipB transfer (~6.5us) has landed.
- A 10-matmul warmup accumulation group (no intermediate semaphores, so it
  runs back-to-back) on a memset tile keeps the PE HAM activity window busy
  from ~5us so the clock gate reaches 2.4 GHz before the real matmuls;
  otherwise the first ~18us run at 1.2 GHz.
- DMA trigger instructions cost ~600ns of sequencer time each, so psi/rt
  loads are batched (one trigger per quarter; q0's psi is split so the very
  first matmul gates on a 128KB slice, with the tail slice on the idle
  SWDGE ring).
- res accumulates fp32 in SBUF; the final (q3) add writes a bf16 staging
  tile DMAed out per 512-col chunk on the scalar ring (idle at the tail).
  The last group is split into two 256-col chunks to shorten the
  post-last-matmul tail.
"""

import numpy as np
import ml_dtypes

import concourse.bacc as bacc
import concourse.mybir as mybir
import concourse.tile as tile
from concourse import bass_utils

F32 = mybir.dt.float32
BF16 = mybir.dt.bfloat16
BF_NP = ml_dtypes.bfloat16

CHI = 1024
W = 5
D = 2
NCORES = 8
H = CHI // NCORES  # 128, h rows per core
NPACK = 43  # 6-g packs per 256-g quarter: 42 full + one 4-g tail

_nc_cache = None


def _build_nc():
    nc = bacc.Bacc("TRN2", target_bir_lowering=False)
    # host-prearranged: psi[ac, q, a_lo, ce, g256]; lt[b, a_lo, ac, h]; rt[blk, g_lo, f, k]
    psi = nc.dram_tensor("psi", [8, 4, 128, 4, 256], BF16, kind="ExternalInput")
    lt = nc.dram_tensor("lt", [5, 128, 8, H], BF16, kind="ExternalInput")
    rt = nc.dram_tensor("rt", [8, 128, 5, 1024], BF16, kind="ExternalInput")
    q6 = nc.dram_tensor("q6", [120, 128], BF16, kind="ExternalInput")
    q4 = nc.dram_tensor("q4", [80, 128], BF16, kind="ExternalInput")
    idn = nc.dram_tensor("idn", [128, 128], BF16, kind="ExternalInput")
    res = nc.dram_tensor("res", [H, 4096], BF16, kind="ExternalOutput")  # h;(i,j,k)

    with tile.TileContext(nc) as tc:
        with (
            tc.tile_pool(name="const", bufs=1) as const_pool,
            tc.tile_pool(name="psis", bufs=2) as psi_pool,
            tc.tile_pool(name="t1", bufs=2) as t1_pool,
            tc.tile_pool(name="t1p", bufs=2) as t1p_pool,
            tc.tile_pool(name="t3h", bufs=2) as t3h_pool,
            tc.tile_pool(name="t3g", bufs=2) as t3g_pool,
            tc.tile_pool(name="rt_p", bufs=2) as rt_pool,
            tc.tile_pool(name="resp", bufs=1) as res_pool,
            tc.tile_pool(name="ps_s1", bufs=2, space="PSUM") as ps_s1,
            tc.tile_pool(name="ps_mid", bufs=2, space="PSUM") as ps_mid,
            tc.tile_pool(name="ps_s4", bufs=2, space="PSUM") as ps_s4,
        ):
            # ---- PE warmup: one accumulation group (no inter-matmul
            # semaphores -> truly back-to-back) so the HAM clock gate opens
            # (1.2 -> 2.4 GHz) before the real matmuls start
            warm_src = const_pool.tile([128, 512], BF16)
            nc.vector.memset(warm_src[:], 1.0)
            wps = ps_s4.tile([128, 512], F32, tag="s4")
            NWARM = 10
            for i in range(NWARM):
                nc.tensor.matmul(
                    wps[:], warm_src[:, 0:128], warm_src[:],
                    start=(i == 0), stop=(i == NWARM - 1),
                )

            # ---- static loads (only b=0 weights gate the first matmul) ----
            lt_sb = const_pool.tile([128, 5, 8, H], BF16)  # [a_lo; b, ac, h]
            lt_r = lt.ap().rearrange("b p ac h -> p b ac h")
            nc.scalar.dma_start(lt_sb[:, 0, 0], lt_r[:, 0, 0])
            nc.scalar.dma_start(lt_sb[:, 0, 1:8], lt_r[:, 0, 1:8])
            q6_sb = const_pool.tile([120, 128], BF16)
            q4_sb = const_pool.tile([80, 128], BF16)
            idn_sb = const_pool.tile([128, 128], BF16)
            res_sb = res_pool.tile([128, 4096], F32)
            res_bf = res_pool.tile([128, 4096], BF16)

            def load_rest_of_consts():
                for b in range(1, 5):
                    nc.scalar.dma_start(lt_sb[:, b], lt_r[:, b])
                nc.scalar.dma_start(q6_sb[:], q6.ap())
                nc.scalar.dma_start(q4_sb[:], q4.ap())
                nc.scalar.dma_start(idn_sb[:], idn.ap())

            evac_ct = 0

            def evac_copy(out, in_):
                # DVE is ~2x faster than ACT for copies; give ACT 1 in 3.
                nonlocal evac_ct
                evac_ct += 1
                if evac_ct % 3 == 0:
                    nc.scalar.copy(out, in_)
                else:
                    nc.vector.tensor_copy(out, in_)

            # deferred step-4 work: list of closures (one per psum group)
            pending_s4 = []

            def fire_s4(n=1):
                for _ in range(n):
                    if pending_s4:
                        pending_s4.pop(0)()

            psi_r = psi.ap()

            def load_psi(qq, gate=None):
                # one SBUF tile per quarter: [a_lo; ac, ce, g256]
                pt = psi_pool.tile([128, 8, 4, 256], BF16, tag="psi")
                if gate is not None:
                    # tiny copy from the gate region into the DMA dest: the
                    # transfer then cannot start before the gate is written,
                    # so this prefetch does not steal HBM bandwidth from the
                    # load the PE is currently consuming (startup only)
                    nc.gpsimd.tensor_copy(pt[0:2, 0, 0, 0:2], gate)
                if qq == 0:
                    # split so the first matmuls gate on small slices; the
                    # ac6-7 tail rides the otherwise-idle SWDGE ring
                    nc.sync.dma_start(pt[:, 0, 0:2], psi_r[0, 0, :, 0:2])
                    nc.sync.dma_start(pt[:, 0, 2:4], psi_r[0, 0, :, 2:4])
                    nc.sync.dma_start(
                        pt[:, 1:3], psi_r[1:3, 0].rearrange("ac p ce g -> p ac ce g")
                    )
                    nc.sync.dma_start(
                        pt[:, 3:6], psi_r[3:6, 0].rearrange("ac p ce g -> p ac ce g")
                    )
                    nc.gpsimd.dma_start(
                        pt[:, 6:8], psi_r[6:8, 0].rearrange("ac p ce g -> p ac ce g")
                    )
                else:
                    nc.sync.dma_start(
                        pt[:], psi_r[:, qq].rearrange("ac p ce g -> p ac ce g")
                    )
                return pt

            def load_rt(qq, gate=None):
                # both 128-g blocks of the quarter in one trigger
                rt_t = rt_pool.tile([128, 2, 5, 1024], BF16, tag="rt")
                if gate is not None:
                    nc.gpsimd.tensor_copy(rt_t[0:2, 0, 0, 0:2], gate)
                nc.sync.dma_start(
                    rt_t[:],
                    rt.ap()[qq * 2 : qq * 2 + 2].rearrange("blk p f k -> p blk f k"),
                )
                return rt_t

            def alloc_t1q():
                t1q = t1_pool.tile([128, NPACK, 128], BF16, tag="t1q")
                # packed destination views: col = g6*20 + (b*4 + ce)
                dstA = t1q[:, 0:42, 0:120].rearrange("p n (g c) -> p n g c", c=20)
                dstB = t1q[:, 42, 0:80].rearrange("p (g c) -> p g c", c=20)
                return {"t1q": t1q, "dstA": dstA, "dstB": dstB}

            def step1_b(st, psi_t, b):
                ps1 = ps_s1.tile([128, 4, 256], F32, tag="s1")  # 2 banks
                ps1_flat = ps1[:].rearrange("p c g -> p (c g)")
                for ac in range(8):
                    lhsT = lt_sb[:, b, ac]
                    psi_flat = psi_t[:, ac].rearrange("p c g -> p (c g)")
                    for cep in range(2):  # one 512-wide MM per PSUM bank
                        nc.tensor.matmul(
                            ps1_flat[:, cep * 512 : (cep + 1) * 512],
                            lhsT,
                            psi_flat[:, cep * 512 : (cep + 1) * 512],
                            start=(ac == 0),
                            stop=(ac == 7),
                        )
                nc.vector.tensor_copy(
                    st["dstA"][:, :, :, b * 4 : b * 4 + 2],
                    ps1[:, 0:2, 0:252].rearrange("p c (n g) -> p n g c", g=6),
                )
                nc.scalar.copy(
                    st["dstA"][:, :, :, b * 4 + 2 : b * 4 + 4],
                    ps1[:, 2:4, 0:252].rearrange("p c (n g) -> p n g c", g=6),
                )
                nc.vector.tensor_copy(
                    st["dstB"][:, :, b * 4 : b * 4 + 2],
                    ps1[:, 0:2, 252:256].rearrange("p c g -> p g c"),
                )
                nc.scalar.copy(
                    st["dstB"][:, :, b * 4 + 2 : b * 4 + 4],
                    ps1[:, 2:4, 252:256].rearrange("p c g -> p g c"),
                )

            # ---------- step 4: defer per-(ij,kh) groups into q+1's body
            last_ps4 = [None]

            def make_s4(qq, t3g_, rt_t):
                def emit(ij, kh, k0, k1):
                    ps4 = ps_s4.tile([128, 512], F32, tag="s4")  # 1 bank
                    last_ps4[0] = ps4
                    kw = k1 - k0
                    for blk2 in range(2):
                        for f in range(5):
                            nc.tensor.matmul(
                                ps4[:, 0:kw],
                                t3g_[:, (ij * 5 + f) * 2 + blk2, :],
                                rt_t[:, blk2, f, kh * 512 + k0 : kh * 512 + k1],
                                start=(blk2 == 0 and f == 0),
                                stop=(blk2 == 1 and f == 4),
                            )
                    c0 = ij * 1024 + kh * 512 + k0
                    if qq == 0:
                        evac_copy(res_sb[:, c0 : c0 + kw], ps4[:, 0:kw])
                    elif qq < 3:
                        nc.vector.tensor_add(
                            res_sb[:, c0 : c0 + kw],
                            res_sb[:, c0 : c0 + kw],
                            ps4[:, 0:kw],
                        )
                    else:
                        # final quarter: add into the bf16 staging tile and
                        # ship it out on the scalar ring (idle at the tail)
                        nc.vector.tensor_add(
                            res_bf[:, c0 : c0 + kw],
                            res_sb[:, c0 : c0 + kw],
                            ps4[:, 0:kw],
                        )
                        nc.scalar.dma_start(
                            res.ap()[:, c0 : c0 + kw], res_bf[:, c0 : c0 + kw]
                        )

                groups = []
                for ij in range(4):
                    for kh in range(2):
                        if qq == 3 and ij == 3 and kh == 1:
                            # split the very last group so the final
                            # accumulate+DMA covers only 256 columns
                            groups.append(lambda ij=ij, kh=kh: emit(ij, kh, 0, 256))
                            groups.append(lambda ij=ij, kh=kh: emit(ij, kh, 256, 512))
                        else:
                            groups.append(lambda ij=ij, kh=kh: emit(ij, kh, 0, 512))
                return groups

            st_by_q = {}
            psis_by_q = {}
            rt_by_q = {}
            for q in range(4):  # g-quarters
                # ---------- step 1: T1q[h; pack, (g6, bce)] ----------
                if q == 0:
                    psis_by_q[0] = load_psi(0)
                    st_by_q[0] = alloc_t1q()
                    load_rest_of_consts()
                # at startup, gate the q1/rt0 prefetches on step1(q0)'s b2
                # output so psi(q0) gets the full HBM pipe first.  For later
                # quarters, gate rt(q) on t3g(q-1): the sync ring is FIFO, so
                # this dispatches rt AFTER flipB(q-1) -- the ring order must
                # be [flipA(q-1), psi(q+1), flipB(q-1), rt(q)] for flipB to
                # land before its consumers (t3g is on the critical path,
                # rt is not).
                g0 = (
                    st_by_q[0]["t1q"][0:2, 0, 8:10] if q == 0 else None
                )
                if q < 3:
                    psis_by_q[q + 1] = load_psi(q + 1, gate=g0)
                rt_gate = g0 if q == 0 else t3g_prev[0:2, 0, 0:2]
                rt_by_q[q] = load_rt(q, gate=rt_gate)
                # leading b-blocks of this quarter were hoisted into the
                # previous body (3 blocks into q0's flipA window, else 2)
                for b in range(0 if q == 0 else (3 if q == 1 else 2), 5):
                    step1_b(st_by_q[q], psis_by_q[q], b)

                # ---------- flipA: DMA XBAR transpose to T1P[(g6,bce); pack, h]
                # ALL XBAR transposes stay on the sync ring: two transposes
                # in flight from different rings corrupt each other (the
                # framework's serialization guard does not cover the
                # cross-ring case -- observed as 15-of-every-16 h rows
                # garbage, the XBAR tile height).
                t1q_flat = st_by_q[q]["t1q"][:].rearrange("p n c -> p (n c)")
                t1p = t1p_pool.tile([128, NPACK, 128], BF16, tag="t1p")
                with tc.high_priority():
                    nc.sync.dma_start(t1p[:], t1q_flat[:], transpose=True)
                # hoist the next quarter's first two step-1 blocks here: they
                # keep the PE busy while the flipA XBAR transpose lands
                if q < 3:
                    st_by_q[q + 1] = alloc_t1q()
                    step1_b(st_by_q[q + 1], psis_by_q[q + 1], 0)
                    fire_s4(1)
                    step1_b(st_by_q[q + 1], psis_by_q[q + 1], 1)
                    fire_s4(1)
                    if q == 0:  # q0 has no deferred step-4 filler
                        step1_b(st_by_q[1], psis_by_q[1], 2)
                    fire_s4(1)
                else:
                    fire_s4(3)
                fire_s4(1)
                # Force mix(q) to sequence AFTER the window work in the PE
                # stream: the scheduler's cost model underestimates the flipA
                # transfer and otherwise places mix's first matmuls (which
                # wait on flipA) ahead of the ready fires -- head-of-line
                # blocking that idles the PE and re-throttles the HAM clock
                # gate.  A bypass-op rewrite of a sliver of every t1p pack
                # (out = in0 = the t1p data itself, in1 = the last fire's
                # PSUM as a pure dependency) makes every mix matmul order
                # after the cover.
                if q > 0 and last_ps4[0] is not None:
                    nc.vector.tensor_tensor(
                        t1p[0:2, :, 0:2],
                        t1p[0:2, :, 0:2],
                        last_ps4[0][0:2, 0:86].rearrange("p (n c) -> p n c", c=2),
                        op=mybir.AluOpType.bypass,
                    )

                # ---------- mix: T3H[h; ijf20, g256] ----------
                t3h = t3h_pool.tile([128, 20, 256], BF16, tag="t3h")
                groups = [list(range(g0, min(g0 + 4, NPACK))) for g0 in range(0, NPACK, 4)]
                for gi, grp in enumerate(groups):
                    pmq = ps_mid.tile([128, 4, 128], F32, tag="mid")  # 1 bank
                    for k, pack in enumerate(grp):
                        if pack < 42:
                            nc.tensor.matmul(
                                pmq[:, k, :], t1p[0:120, pack, :], q6_sb[:],
                                start=True, stop=True,
                            )
                        else:
                            nc.tensor.matmul(
                                pmq[:, k, :], t1p[0:80, 42, :], q4_sb[:],
                                start=True, stop=True,
                            )
                    nfull = sum(1 for p_ in grp if p_ < 42)
                    if nfull:
                        evac_copy(
                            t3h[:, :, grp[0] * 6 : grp[0] * 6 + nfull * 6].rearrange(
                                "p i (k g) -> p k i g", g=6
                            ),
                            pmq[:, 0:nfull, 0:120].rearrange(
                                "p k (i g) -> p k i g", g=6
                            ),
                        )
                    if grp[-1] == 42:
                        evac_copy(
                            t3h[:, :, 252:256],
                            pmq[:, nfull, 0:80].rearrange("p (i g) -> p i g", g=4),
                        )
                    if gi in (2, 5):
                        fire_s4(1)
                fire_s4(2)

                # ---------- flipB: T3G[g; (ijf, blk), h] ----------
                t3g = t3g_pool.tile([128, 40, 128], BF16, tag="t3g")
                t3g_prev = t3g
                if q < 3:
                    # one XBAR transpose: every 128-col chunk of flat T3H is
                    # exactly one [h; g128] tile for (ijf = n//2, blk = n%2)
                    t3h_flat = t3h[:].rearrange("p i g -> p (i g)")
                    with tc.high_priority():
                        nc.sync.dma_start(t3g[:], t3h_flat, transpose=True)
                    pending_s4.extend(make_s4(q, t3g, rt_by_q[q]))
                else:
                    # last quarter: keep flipB on the PE so its step-4 groups
                    # interleave in as soon as their t3g slices are ready
                    pending_s4.extend(make_s4(q, t3g, rt_by_q[q]))
                    for ng in range(10):  # 4 transposes per PSUM bank
                        pb = ps_mid.tile([128, 4, 128], BF16, tag="mid")
                        for j in range(4):
                            n = ng * 4 + j  # n = ijf*2 + blk2
                            nc.tensor.transpose(
                                pb[:, j, :],
                                t3h[:, n // 2, (n % 2) * 128 : (n % 2) * 128 + 128],
                                idn_sb[:],
                            )
                        evac_copy(
                            t3g[:, ng * 4 : (ng + 1) * 4, :].rearrange(
                                "p n h -> p (n h)"
                            ),
                            pb[:].rearrange("p j h -> p (j h)"),
                        )
                        if ng in (3, 6):
                            fire_s4(2)  # ij0 after n<=15 done, ij1 after n<=27

            # flush remaining deferred step-4 work (last quarter)
            fire_s4(len(pending_s4))
    nc.compile()
    return nc


def _host_inputs(psi_flat, L, M1, M2, R):
    # psi[a,ce,g] -> [ac, q, a_lo, ce, g256]
    psi = np.ascontiguousarray(
        psi_flat.reshape(8, 128, 4, 4, 256).transpose(0, 3, 1, 2, 4)
    ).astype(BF_NP)
    # R[f,k,g] -> RT[f,g,k] -> [blk, g_lo, f, k]
    RT = np.ascontiguousarray(
        R.transpose(2, 0, 1).reshape(8, 128, 5, 1024)
    ).astype(BF_NP)
    Q = np.einsum("bdic,dfje->bceijf", M1, M2).reshape(20, 20).astype(np.float32)
    rows = np.arange(20)
    Q6P = np.zeros((120, 128), np.float32)
    for g6 in range(6):
        Q6P[np.ix_(g6 * 20 + rows, rows * 6 + g6)] = Q
    Q4P = np.zeros((80, 128), np.float32)
    for g4 in range(4):
        Q4P[np.ix_(g4 * 20 + rows, rows * 4 + g4)] = Q
    Q6P = Q6P.astype(BF_NP)
    Q4P = Q4P.astype(BF_NP)
    idn = np.eye(128, dtype=np.float32).astype(BF_NP)
    in_maps = []
    for c in range(NCORES):
        LT = np.ascontiguousarray(
            L[:, c * H : (c + 1) * H, :].transpose(0, 2, 1).reshape(5, 8, 128, H)
            .transpose(0, 2, 1, 3)
        ).astype(BF_NP)  # [b, a_lo, ac, h]
        in_maps.append({"psi": psi, "lt": LT, "rt": RT, "q6": Q6P, "q4": Q4P, "idn": idn})
    return in_maps


def kernel(**inputs):
    psi_flat = np.asarray(inputs["psi_flat"], np.float32)
    L = np.asarray(inputs["L"], np.float32)
    M1 = np.asarray(inputs["M1"], np.float32)
    M2 = np.asarray(inputs["M2"], np.float32)
    R = np.asarray(inputs["R"], np.float32)

    global _nc_cache
    if _nc_cache is None:
        _nc_cache = _build_nc()
    nc = _nc_cache

    in_maps = _host_inputs(psi_flat, L, M1, M2, R)
    out = bass_utils.run_bass_kernel_spmd(nc, in_maps, core_ids=list(range(NCORES)))
    parts = [
        np.asarray(out.results[c]["res"]).astype(np.float32) for c in range(NCORES)
    ]
    return np.concatenate(parts, axis=0).reshape(-1)


# revision 34
# speedup vs baseline: 1.0965x; 1.0110x over previous
"""DMRG two-site effective Hamiltonian application (ApplyMPO) on 8 trn2 cores.

Math (reference):
  res[h,i,j,k] = sum_{a,b,c,d,e,f,g} L[b,h,a] M1[b,d,i,c] M2[d,f,j,e]
                                     R[f,k,g] psi[a,c,e,g]

Device algorithm (per core, output bond h sharded 8 x 128), all bf16 with
fp32 PSUM accumulation:
  Q[(b,c,e),(i,j,f)] = sum_d M1[b,d,i,c] M2[d,f,j,e]            (host, 400 els)
  step1: T1[h; pack, (g6,bce)] = sum_a L[b,h,a] psi[a,(c,e),g]  (PE, K=a)
         written in 128-col packs: col = g6*20 + (b*4+ce), 6 g's per pack
  flipA: T1P[(g6,bce); pack, h] = DMA XBAR transpose of T1       (DMA engine)
  mix:   T3H[h; (i,j,f), g]     = T1P[pack]^T @ Q6P              (PE, K=20 eff)
  flipB: T3G[g; (ijf,blk), h]   = one DMA XBAR transpose of T3H  (q<3; q3: PE)
  step4: res[h; ij,k]          += T3G[ijf,blk]^T @ R^T[f][g,k]   (PE, K=g, acc f)

Scheduling notes (from trace analysis):
- PE is the bottleneck (~85% busy).  flipB rides the XBAR for q<3 (one
  DmaTransposeAnt per quarter: each 128-col chunk of flat T3H is exactly one
  [h; g128] tile); q3 keeps flipB on the PE so its step-4 groups can
  interleave into the flipB loop at the tail.
- flipA stays split across the two HWDGE rings as in the tuned baseline;
  flipB(q) goes on the sync ring after the flipA halves.  Consumers of
  t3g(q) fire no earlier than b4 of step1(q+1), ~14us after mix(q), by
  which time the flipB transfer (~6.5us) has landed.
- A 10-matmul warmup accumulation group (no intermediate semaphores, so it
  runs back-to-back) on a memset tile keeps the PE HAM activity window busy
  from ~5us so the clock gate reaches 2.4 GHz before the real matmuls;
  otherwise the first ~18us run at 1.2 GHz.
- DMA trigger instructions cost ~600ns of sequencer time each, so psi/rt
  loads are batched (one trigger per quarter; q0's psi is split so the very
  first matmul gates on a 128KB slice, with the tail slice on the idle
  SWDGE ring).
- res accumulates fp32 in SBUF; the final (q3) add writes a bf16 staging
  tile DMAed out per 512-col chunk on the scalar ring (idle at the tail).
  The last group is split into two 256-col chunks to shorten the
  post-last-matmul tail.
"""

import numpy as np
import ml_dtypes

import concourse.bacc as bacc
import concourse.mybir as mybir
import concourse.tile as tile
from concourse import bass_utils

F32 = mybir.dt.float32
BF16 = mybir.dt.bfloat16
BF_NP = ml_dtypes.bfloat16

CHI = 1024
W = 5
D = 2
NCORES = 8
H = CHI // NCORES  # 128, h rows per core
NPACK = 43  # 6-g packs per 256-g quarter: 42 full + one 4-g tail

_nc_cache = None


def _build_nc():
    nc = bacc.Bacc("TRN2", target_bir_lowering=False)
    # host-prearranged: psi[ac, q, a_lo, ce, g256]; lt[b, a_lo, ac, h]; rt[blk, g_lo, f, k]
    psi = nc.dram_tensor("psi", [8, 4, 128, 4, 256], BF16, kind="ExternalInput")
    lt = nc.dram_tensor("lt", [5, 128, 8, H], BF16, kind="ExternalInput")
    rt = nc.dram_tensor("rt", [8, 128, 5, 1024], BF16, kind="ExternalInput")
    q6 = nc.dram_tensor("q6", [120, 128], BF16, kind="ExternalInput")
    q4 = nc.dram_tensor("q4", [80, 128], BF16, kind="ExternalInput")
    idn = nc.dram_tensor("idn", [128, 128], BF16, kind="ExternalInput")
    res = nc.dram_tensor("res", [H, 4096], BF16, kind="ExternalOutput")  # h;(i,j,k)

    with tile.TileContext(nc) as tc:
        with (
            tc.tile_pool(name="const", bufs=1) as const_pool,
            tc.tile_pool(name="psis", bufs=2) as psi_pool,
            tc.tile_pool(name="t1", bufs=2) as t1_pool,
            tc.tile_pool(name="t1p", bufs=2) as t1p_pool,
            tc.tile_pool(name="t3h", bufs=2) as t3h_pool,
            tc.tile_pool(name="t3g", bufs=2) as t3g_pool,
            tc.tile_pool(name="rt_p", bufs=2) as rt_pool,
            tc.tile_pool(name="resp", bufs=1) as res_pool,
            tc.tile_pool(name="ps_s1", bufs=2, space="PSUM") as ps_s1,
            tc.tile_pool(name="ps_mid", bufs=2, space="PSUM") as ps_mid,
            tc.tile_pool(name="ps_s4", bufs=2, space="PSUM") as ps_s4,
        ):
            # ---- PE warmup: one accumulation group (no inter-matmul
            # semaphores -> truly back-to-back) so the HAM clock gate opens
            # (1.2 -> 2.4 GHz) before the real matmuls start
            warm_src = const_pool.tile([128, 512], BF16)
            nc.vector.memset(warm_src[:], 1.0)
            wps = ps_s4.tile([128, 512], F32, tag="s4")
            NWARM = 10
            for i in range(NWARM):
                nc.tensor.matmul(
                    wps[:], warm_src[:, 0:128], warm_src[:],
                    start=(i == 0), stop=(i == NWARM - 1),
                )

            # ---- static loads (only b=0 weights gate the first matmul) ----
            lt_sb = const_pool.tile([128, 5, 8, H], BF16)  # [a_lo; b, ac, h]
            lt_r = lt.ap().rearrange("b p ac h -> p b ac h")
            nc.scalar.dma_start(lt_sb[:, 0, 0], lt_r[:, 0, 0])
            nc.scalar.dma_start(lt_sb[:, 0, 1:8], lt_r[:, 0, 1:8])
            q6_sb = const_pool.tile([120, 128], BF16)
            q4_sb = const_pool.tile([80, 128], BF16)
            idn_sb = const_pool.tile([128, 128], BF16)
            res_sb = res_pool.tile([128, 4096], F32)
            res_bf = res_pool.tile([128, 4096], BF16)

            def load_rest_of_consts():
                for b in range(1, 5):
                    nc.scalar.dma_start(lt_sb[:, b], lt_r[:, b])
                nc.scalar.dma_start(q6_sb[:], q6.ap())
                nc.scalar.dma_start(q4_sb[:], q4.ap())
                nc.scalar.dma_start(idn_sb[:], idn.ap())

            evac_ct = 0

            def evac_copy(out, in_):
                # DVE is ~2x faster than ACT for copies; give ACT 1 in 3.
                nonlocal evac_ct
                evac_ct += 1
                if evac_ct % 3 == 0:
                    nc.scalar.copy(out, in_)
                else:
                    nc.vector.tensor_copy(out, in_)

            # deferred step-4 work: list of closures (one per psum group)
            pending_s4 = []

            def fire_s4(n=1):
                for _ in range(n):
                    if pending_s4:
                        pending_s4.pop(0)()

            psi_r = psi.ap()

            def load_psi(qq, gate=None):
                # one SBUF tile per quarter: [a_lo; ac, ce, g256]
                pt = psi_pool.tile([128, 8, 4, 256], BF16, tag="psi")
                if gate is not None:
                    # tiny copy from the gate region into the DMA dest: the
                    # transfer then cannot start before the gate is written,
                    # so this prefetch does not steal HBM bandwidth from the
                    # load the PE is currently consuming (startup only)
                    nc.gpsimd.tensor_copy(pt[0:2, 0, 0, 0:2], gate)
                if qq == 0:
                    # split so the first matmuls gate on small slices; the
                    # ac6-7 tail rides the otherwise-idle SWDGE ring
                    nc.sync.dma_start(pt[:, 0, 0:2], psi_r[0, 0, :, 0:2])
                    nc.sync.dma_start(pt[:, 0, 2:4], psi_r[0, 0, :, 2:4])
                    nc.sync.dma_start(
                        pt[:, 1:3], psi_r[1:3, 0].rearrange("ac p ce g -> p ac ce g")
                    )
                    nc.sync.dma_start(
                        pt[:, 3:6], psi_r[3:6, 0].rearrange("ac p ce g -> p ac ce g")
                    )
                    nc.gpsimd.dma_start(
                        pt[:, 6:8], psi_r[6:8, 0].rearrange("ac p ce g -> p ac ce g")
                    )
                else:
                    nc.sync.dma_start(
                        pt[:], psi_r[:, qq].rearrange("ac p ce g -> p ac ce g")
                    )
                return pt

            def load_rt(qq, gate=None):
                # both 128-g blocks of the quarter in one trigger
                rt_t = rt_pool.tile([128, 2, 5, 1024], BF16, tag="rt")
                if gate is not None:
                    nc.gpsimd.tensor_copy(rt_t[0:2, 0, 0, 0:2], gate)
                nc.sync.dma_start(
                    rt_t[:],
                    rt.ap()[qq * 2 : qq * 2 + 2].rearrange("blk p f k -> p blk f k"),
                )
                return rt_t

            def alloc_t1q():
                t1q = t1_pool.tile([128, NPACK, 128], BF16, tag="t1q")
                # packed destination views: col = g6*20 + (b*4 + ce)
                dstA = t1q[:, 0:42, 0:120].rearrange("p n (g c) -> p n g c", c=20)
                dstB = t1q[:, 42, 0:80].rearrange("p (g c) -> p g c", c=20)
                return {"t1q": t1q, "dstA": dstA, "dstB": dstB}

            def step1_b(st, psi_t, b):
                ps1 = ps_s1.tile([128, 4, 256], F32, tag="s1")  # 2 banks
                ps1_flat = ps1[:].rearrange("p c g -> p (c g)")
                for ac in range(8):
                    lhsT = lt_sb[:, b, ac]
                    psi_flat = psi_t[:, ac].rearrange("p c g -> p (c g)")
                    for cep in range(2):  # one 512-wide MM per PSUM bank
                        nc.tensor.matmul(
                            ps1_flat[:, cep * 512 : (cep + 1) * 512],
                            lhsT,
                            psi_flat[:, cep * 512 : (cep + 1) * 512],
                            start=(ac == 0),
                            stop=(ac == 7),
                        )
                nc.vector.tensor_copy(
                    st["dstA"][:, :, :, b * 4 : b * 4 + 2],
                    ps1[:, 0:2, 0:252].rearrange("p c (n g) -> p n g c", g=6),
                )
                nc.scalar.copy(
                    st["dstA"][:, :, :, b * 4 + 2 : b * 4 + 4],
                    ps1[:, 2:4, 0:252].rearrange("p c (n g) -> p n g c", g=6),
                )
                nc.vector.tensor_copy(
                    st["dstB"][:, :, b * 4 : b * 4 + 2],
                    ps1[:, 0:2, 252:256].rearrange("p c g -> p g c"),
                )
                nc.scalar.copy(
                    st["dstB"][:, :, b * 4 + 2 : b * 4 + 4],
                    ps1[:, 2:4, 252:256].rearrange("p c g -> p g c"),
                )

            # ---------- step 4: defer per-(ij,kh) groups into q+1's body
            last_ps4 = [None]

            def make_s4(qq, t3g_, rt_t):
                def emit(ij, kh, k0, k1):
                    ps4 = ps_s4.tile([128, 512], F32, tag="s4")  # 1 bank
                    last_ps4[0] = ps4
                    kw = k1 - k0
                    for blk2 in range(2):
                        for f in range(5):
                            nc.tensor.matmul(
                                ps4[:, 0:kw],
                                t3g_[:, (ij * 5 + f) * 2 + blk2, :],
                                rt_t[:, blk2, f, kh * 512 + k0 : kh * 512 + k1],
                                start=(blk2 == 0 and f == 0),
                                stop=(blk2 == 1 and f == 4),
                            )
                    c0 = ij * 1024 + kh * 512 + k0
                    if qq == 0:
                        evac_copy(res_sb[:, c0 : c0 + kw], ps4[:, 0:kw])
                    elif qq < 3:
                        nc.vector.tensor_add(
                            res_sb[:, c0 : c0 + kw],
                            res_sb[:, c0 : c0 + kw],
                            ps4[:, 0:kw],
                        )
                    else:
                        # final quarter: add into the bf16 staging tile and
                        # ship it out on the scalar ring (idle at the tail)
                        nc.vector.tensor_add(
                            res_bf[:, c0 : c0 + kw],
                            res_sb[:, c0 : c0 + kw],
                            ps4[:, 0:kw],
                        )
                        nc.scalar.dma_start(
                            res.ap()[:, c0 : c0 + kw], res_bf[:, c0 : c0 + kw]
                        )

                groups = []
                for ij in range(4):
                    for kh in range(2):
                        if qq == 3 and ij == 3 and kh == 1:
                            # split the very last group so the final
                            # accumulate+DMA covers only 256 columns
                            groups.append(lambda ij=ij, kh=kh: emit(ij, kh, 0, 256))
                            groups.append(lambda ij=ij, kh=kh: emit(ij, kh, 256, 512))
                        else:
                            groups.append(lambda ij=ij, kh=kh: emit(ij, kh, 0, 512))
                return groups

            st_by_q = {}
            psis_by_q = {}
            rt_by_q = {}
            for q in range(4):  # g-quarters
                # ---------- step 1: T1q[h; pack, (g6, bce)] ----------
                if q == 0:
                    psis_by_q[0] = load_psi(0)
                    st_by_q[0] = alloc_t1q()
                    load_rest_of_consts()
                # at startup, gate the q1/rt0 prefetches on step1(q0)'s b2
                # output so psi(q0) gets the full HBM pipe first.  Later rt
                # loads happen a quarter ahead, right after flipB (see
                # below), so the sync-ring FIFO order per quarter is
                # [flipA(q), psi(q+2), flipB(q), rt(q+1)] -- flipB lands
                # before its consumers and rt has a whole quarter of slack.
                g0 = (
                    st_by_q[0]["t1q"][0:2, 0, 8:10] if q == 0 else None
                )
                if q < 3:
                    psis_by_q[q + 1] = load_psi(q + 1, gate=g0)
                if q == 0:
                    rt_by_q[0] = load_rt(0, gate=g0)
                # leading b-blocks of this quarter were hoisted into the
                # previous body (3 blocks into q0's flipA window, else 2)
                for b in range(0 if q == 0 else (3 if q == 1 else 2), 5):
                    step1_b(st_by_q[q], psis_by_q[q], b)

                # ---------- flipA: DMA XBAR transpose to T1P[(g6,bce); pack, h]
                # ALL XBAR transposes stay on the sync ring: two transposes
                # in flight from different rings corrupt each other (the
                # framework's serialization guard does not cover the
                # cross-ring case -- observed as 15-of-every-16 h rows
                # garbage, the XBAR tile height).
                t1q_flat = st_by_q[q]["t1q"][:].rearrange("p n c -> p (n c)")
                t1p = t1p_pool.tile([128, NPACK, 128], BF16, tag="t1p")
                with tc.high_priority():
                    nc.sync.dma_start(t1p[:], t1q_flat[:], transpose=True)
                # hoist the next quarter's first two step-1 blocks here: they
                # keep the PE busy while the flipA XBAR transpose lands
                if q < 3:
                    st_by_q[q + 1] = alloc_t1q()
                    step1_b(st_by_q[q + 1], psis_by_q[q + 1], 0)
                    fire_s4(1)
                    step1_b(st_by_q[q + 1], psis_by_q[q + 1], 1)
                    fire_s4(1)
                    if q == 0:  # q0 has no deferred step-4 filler
                        step1_b(st_by_q[1], psis_by_q[1], 2)
                    fire_s4(1)
                else:
                    fire_s4(3)
                fire_s4(1)
                # Force mix(q) to sequence AFTER the window work in the PE
                # stream: the scheduler's cost model underestimates the flipA
                # transfer and otherwise places mix's first matmuls (which
                # wait on flipA) ahead of the ready fires -- head-of-line
                # blocking that idles the PE and re-throttles the HAM clock
                # gate.  A bypass-op rewrite of a sliver of every t1p pack
                # (out = in0 = the t1p data itself, in1 = the last fire's
                # PSUM as a pure dependency) makes every mix matmul order
                # after the cover.
                if q > 0 and last_ps4[0] is not None:
                    nc.vector.tensor_tensor(
                        t1p[0:2, :, 0:2],
                        t1p[0:2, :, 0:2],
                        last_ps4[0][0:2, 0:86].rearrange("p (n c) -> p n c", c=2),
                        op=mybir.AluOpType.bypass,
                    )

                # ---------- mix: T3H[h; ijf20, g256] ----------
                t3h = t3h_pool.tile([128, 20, 256], BF16, tag="t3h")
                groups = [list(range(g0, min(g0 + 4, NPACK))) for g0 in range(0, NPACK, 4)]
                for gi, grp in enumerate(groups):
                    pmq = ps_mid.tile([128, 4, 128], F32, tag="mid")  # 1 bank
                    for k, pack in enumerate(grp):
                        if pack < 42:
                            nc.tensor.matmul(
                                pmq[:, k, :], t1p[0:120, pack, :], q6_sb[:],
                                start=True, stop=True,
                            )
                        else:
                            nc.tensor.matmul(
                                pmq[:, k, :], t1p[0:80, 42, :], q4_sb[:],
                                start=True, stop=True,
                            )
                    nfull = sum(1 for p_ in grp if p_ < 42)
                    if nfull:
                        evac_copy(
                            t3h[:, :, grp[0] * 6 : grp[0] * 6 + nfull * 6].rearrange(
                                "p i (k g) -> p k i g", g=6
                            ),
                            pmq[:, 0:nfull, 0:120].rearrange(
                                "p k (i g) -> p k i g", g=6
                            ),
                        )
                    if grp[-1] == 42:
                        evac_copy(
                            t3h[:, :, 252:256],
                            pmq[:, nfull, 0:80].rearrange("p (i g) -> p i g", g=4),
                        )
                    if gi in (2, 5):
                        fire_s4(1)
                fire_s4(2)

                # ---------- flipB: T3G[g; (ijf, blk), h] ----------
                t3g = t3g_pool.tile([128, 40, 128], BF16, tag="t3g")
                if q < 3:
                    # one XBAR transpose: every 128-col chunk of flat T3H is
                    # exactly one [h; g128] tile for (ijf = n//2, blk = n%2)
                    t3h_flat = t3h[:].rearrange("p i g -> p (i g)")
                    with tc.high_priority():
                        nc.sync.dma_start(t3g[:], t3h_flat, transpose=True)
                    # prefetch rt(q+1) a quarter ahead, gated on t3g(q) so
                    # its transfer queues AFTER the flipB XBAR on the ring
                    rt_by_q[q + 1] = load_rt(q + 1, gate=t3g[0:2, 0, 0:2])
                    pending_s4.extend(make_s4(q, t3g, rt_by_q[q]))
                else:
                    # last quarter: keep flipB on the PE so its step-4 groups
                    # interleave in as soon as their t3g slices are ready
                    pending_s4.extend(make_s4(q, t3g, rt_by_q[q]))
                    for ng in range(10):  # 4 transposes per PSUM bank
                        pb = ps_mid.tile([128, 4, 128], BF16, tag="mid")
                        for j in range(4):
                            n = ng * 4 + j  # n = ijf*2 + blk2
                            nc.tensor.transpose(
                                pb[:, j, :],
                                t3h[:, n // 2, (n % 2) * 128 : (n % 2) * 128 + 128],
                                idn_sb[:],
                            )
                        evac_copy(
                            t3g[:, ng * 4 : (ng + 1) * 4, :].rearrange(
                                "p n h -> p (n h)"
                            ),
                            pb[:].rearrange("p j h -> p (j h)"),
                        )
                        if ng in (3, 6):
                            fire_s4(2)  # ij0 after n<=15 done, ij1 after n<=27

            # flush remaining deferred step-4 work (last quarter)
            fire_s4(len(pending_s4))
    nc.compile()
    return nc


def _host_inputs(psi_flat, L, M1, M2, R):
    # psi[a,ce,g] -> [ac, q, a_lo, ce, g256]
    psi = np.ascontiguousarray(
        psi_flat.reshape(8, 128, 4, 4, 256).transpose(0, 3, 1, 2, 4)
    ).astype(BF_NP)
    # R[f,k,g] -> RT[f,g,k] -> [blk, g_lo, f, k]
    RT = np.ascontiguousarray(
        R.transpose(2, 0, 1).reshape(8, 128, 5, 1024)
    ).astype(BF_NP)
    Q = np.einsum("bdic,dfje->bceijf", M1, M2).reshape(20, 20).astype(np.float32)
    rows = np.arange(20)
    Q6P = np.zeros((120, 128), np.float32)
    for g6 in range(6):
        Q6P[np.ix_(g6 * 20 + rows, rows * 6 + g6)] = Q
    Q4P = np.zeros((80, 128), np.float32)
    for g4 in range(4):
        Q4P[np.ix_(g4 * 20 + rows, rows * 4 + g4)] = Q
    Q6P = Q6P.astype(BF_NP)
    Q4P = Q4P.astype(BF_NP)
    idn = np.eye(128, dtype=np.float32).astype(BF_NP)
    in_maps = []
    for c in range(NCORES):
        LT = np.ascontiguousarray(
            L[:, c * H : (c + 1) * H, :].transpose(0, 2, 1).reshape(5, 8, 128, H)
            .transpose(0, 2, 1, 3)
        ).astype(BF_NP)  # [b, a_lo, ac, h]
        in_maps.append({"psi": psi, "lt": LT, "rt": RT, "q6": Q6P, "q4": Q4P, "idn": idn})
    return in_maps


def kernel(**inputs):
    psi_flat = np.asarray(inputs["psi_flat"], np.float32)
    L = np.asarray(inputs["L"], np.float32)
    M1 = np.asarray(inputs["M1"], np.float32)
    M2 = np.asarray(inputs["M2"], np.float32)
    R = np.asarray(inputs["R"], np.float32)

    global _nc_cache
    if _nc_cache is None:
        _nc_cache = _build_nc()
    nc = _nc_cache

    in_maps = _host_inputs(psi_flat, L, M1, M2, R)
    out = bass_utils.run_bass_kernel_spmd(nc, in_maps, core_ids=list(range(NCORES)))
    parts = [
        np.asarray(out.results[c]["res"]).astype(np.float32) for c in range(NCORES)
    ]
    return np.concatenate(parts, axis=0).reshape(-1)


# revision 35
# speedup vs baseline: 1.1177x; 1.0194x over previous
"""DMRG two-site effective Hamiltonian application (ApplyMPO) on 8 trn2 cores.

Math (reference):
  res[h,i,j,k] = sum_{a,b,c,d,e,f,g} L[b,h,a] M1[b,d,i,c] M2[d,f,j,e]
                                     R[f,k,g] psi[a,c,e,g]

Device algorithm (per core, output bond h sharded 8 x 128), all bf16 with
fp32 PSUM accumulation:
  Q[(b,c,e),(i,j,f)] = sum_d M1[b,d,i,c] M2[d,f,j,e]            (host, 400 els)
  step1: T1[h; pack, (g6,bce)] = sum_a L[b,h,a] psi[a,(c,e),g]  (PE, K=a)
         written in 128-col packs: col = g6*20 + (b*4+ce), 6 g's per pack
  flipA: T1P[(g6,bce); pack, h] = DMA XBAR transpose of T1       (DMA engine)
  mix:   T3H[h; (i,j,f), g]     = T1P[pack]^T @ Q6P              (PE, K=20 eff)
  flipB: T3G[g; (ijf,blk), h]   = PE transpose of T3H g-slices   (PE, 1 c/r)
  step4: res[h; ij,k]          += T3G[ijf,blk]^T @ R^T[f][g,k]   (PE, K=g, acc f)
flipA rides the DMA XBAR whose latency is hidden behind hoisted step-1 work;
flipB stays on the PE: moving it to the XBAR was tried and loses ~8us to
ring-FIFO serialization (flipA + prefetches + flipB all share the sync ring,
and two XBAR transposes in flight from different rings corrupt each other).

Scheduling notes (from trace analysis):
- A 10-matmul warmup accumulation group (one PSUM group -> no intermediate
  semaphores -> truly back-to-back) on a memset tile runs before any data
  lands so the PE HAM clock gate reaches 2.4 GHz (~3.4us of sustained
  activity) before the real matmuls; otherwise the first ~18us run at
  1.2 GHz.
- DMA trigger instructions cost ~600ns of sequencer time each, so psi/rt
  loads are batched (one trigger per quarter; q0's psi is split so the very
  first matmul gates on a 128KB slice, with the ac6-7 tail on the idle
  SWDGE ring for parallel transfer).
- psi(q1)/rt(q0) prefetches are gated on step1(q0)'s b2 output via a tiny
  SWDGE copy into their destination tiles, so the startup psi(q0) load gets
  the full HBM pipe (otherwise the prefetch transfers halve its bandwidth
  and the PE starves ~5us while HAM re-throttles).
- res accumulates fp32 in SBUF; the final (q3) add writes a bf16 staging
  tile DMAed out per 512-col chunk on the scalar ring (idle at the tail).
  The last group is split into two 256-col chunks to shorten the
  post-last-matmul tail.
"""

import numpy as np
import ml_dtypes

import concourse.bacc as bacc
import concourse.mybir as mybir
import concourse.tile as tile
from concourse import bass_utils

F32 = mybir.dt.float32
BF16 = mybir.dt.bfloat16
BF_NP = ml_dtypes.bfloat16

CHI = 1024
W = 5
D = 2
NCORES = 8
H = CHI // NCORES  # 128, h rows per core
NPACK = 43  # 6-g packs per 256-g quarter: 42 full + one 4-g tail

_nc_cache = None


def _build_nc():
    nc = bacc.Bacc("TRN2", target_bir_lowering=False)
    # host-prearranged: psi[ac, q, a_lo, ce, g256]; lt[b, a_lo, ac, h]; rt[blk, g_lo, f, k]
    psi = nc.dram_tensor("psi", [8, 4, 128, 4, 256], BF16, kind="ExternalInput")
    lt = nc.dram_tensor("lt", [5, 128, 8, H], BF16, kind="ExternalInput")
    rt = nc.dram_tensor("rt", [8, 128, 5, 1024], BF16, kind="ExternalInput")
    q6 = nc.dram_tensor("q6", [120, 128], BF16, kind="ExternalInput")
    q4 = nc.dram_tensor("q4", [80, 128], BF16, kind="ExternalInput")
    idn = nc.dram_tensor("idn", [128, 128], BF16, kind="ExternalInput")
    res = nc.dram_tensor("res", [H, 4096], BF16, kind="ExternalOutput")  # h;(i,j,k)

    with tile.TileContext(nc) as tc:
        with (
            tc.tile_pool(name="const", bufs=1) as const_pool,
            tc.tile_pool(name="psis", bufs=2) as psi_pool,
            tc.tile_pool(name="t1", bufs=2) as t1_pool,
            tc.tile_pool(name="t1p", bufs=2) as t1p_pool,
            tc.tile_pool(name="t3h", bufs=2) as t3h_pool,
            tc.tile_pool(name="t3g", bufs=2) as t3g_pool,
            tc.tile_pool(name="rt_p", bufs=2) as rt_pool,
            tc.tile_pool(name="resp", bufs=1) as res_pool,
            tc.tile_pool(name="ps_s1", bufs=2, space="PSUM") as ps_s1,
            tc.tile_pool(name="ps_mid", bufs=2, space="PSUM") as ps_mid,
            tc.tile_pool(name="ps_s4", bufs=2, space="PSUM") as ps_s4,
        ):
            # ---- PE warmup ----
            warm_src = const_pool.tile([128, 512], BF16)
            nc.vector.memset(warm_src[:], 1.0)
            wps = ps_s4.tile([128, 512], F32, tag="s4")
            NWARM = 10
            for i in range(NWARM):
                nc.tensor.matmul(
                    wps[:], warm_src[:, 0:128], warm_src[:],
                    start=(i == 0), stop=(i == NWARM - 1),
                )

            # ---- static loads (only b=0 weights gate the first matmul) ----
            lt_sb = const_pool.tile([128, 5, 8, H], BF16)  # [a_lo; b, ac, h]
            lt_r = lt.ap().rearrange("b p ac h -> p b ac h")
            nc.scalar.dma_start(lt_sb[:, 0, 0], lt_r[:, 0, 0])
            nc.scalar.dma_start(lt_sb[:, 0, 1:8], lt_r[:, 0, 1:8])
            q6_sb = const_pool.tile([120, 128], BF16)
            q4_sb = const_pool.tile([80, 128], BF16)
            idn_sb = const_pool.tile([128, 128], BF16)
            res_sb = res_pool.tile([128, 4096], F32)
            res_bf = res_pool.tile([128, 4096], BF16)

            def load_rest_of_consts():
                for b in range(1, 5):
                    nc.scalar.dma_start(lt_sb[:, b], lt_r[:, b])
                nc.scalar.dma_start(q6_sb[:], q6.ap())
                nc.scalar.dma_start(q4_sb[:], q4.ap())
                nc.scalar.dma_start(idn_sb[:], idn.ap())

            evac_ct = 0

            def evac_copy(out, in_):
                # DVE is ~2x faster than ACT for copies; give ACT 1 in 3.
                nonlocal evac_ct
                evac_ct += 1
                if evac_ct % 3 == 0:
                    nc.scalar.copy(out, in_)
                else:
                    nc.vector.tensor_copy(out, in_)

            # deferred step-4 work: list of closures (one per psum group)
            pending_s4 = []

            def fire_s4(n=1):
                for _ in range(n):
                    if pending_s4:
                        pending_s4.pop(0)()

            psi_r = psi.ap()

            def load_psi(qq, gate=None):
                # one SBUF tile per quarter: [a_lo; ac, ce, g256]
                pt = psi_pool.tile([128, 8, 4, 256], BF16, tag="psi")
                if gate is not None:
                    # tiny copy from the gate region into the DMA dest: the
                    # transfer then cannot start before the gate is written
                    # (startup bandwidth protection)
                    nc.gpsimd.tensor_copy(pt[0:2, 0, 0, 0:2], gate)
                if qq == 0:
                    nc.sync.dma_start(pt[:, 0, 0:2], psi_r[0, 0, :, 0:2])
                    nc.sync.dma_start(pt[:, 0, 2:4], psi_r[0, 0, :, 2:4])
                    nc.sync.dma_start(
                        pt[:, 1:3], psi_r[1:3, 0].rearrange("ac p ce g -> p ac ce g")
                    )
                    nc.sync.dma_start(
                        pt[:, 3:6], psi_r[3:6, 0].rearrange("ac p ce g -> p ac ce g")
                    )
                    nc.gpsimd.dma_start(
                        pt[:, 6:8], psi_r[6:8, 0].rearrange("ac p ce g -> p ac ce g")
                    )
                else:
                    nc.sync.dma_start(
                        pt[:], psi_r[:, qq].rearrange("ac p ce g -> p ac ce g")
                    )
                return pt

            def load_rt(qq, gate=None):
                # both 128-g blocks of the quarter in one trigger
                rt_t = rt_pool.tile([128, 2, 5, 1024], BF16, tag="rt")
                if gate is not None:
                    nc.gpsimd.tensor_copy(rt_t[0:2, 0, 0, 0:2], gate)
                nc.sync.dma_start(
                    rt_t[:],
                    rt.ap()[qq * 2 : qq * 2 + 2].rearrange("blk p f k -> p blk f k"),
                )
                return rt_t

            def alloc_t1q():
                t1q = t1_pool.tile([128, NPACK, 128], BF16, tag="t1q")
                # packed destination views: col = g6*20 + (b*4 + ce)
                dstA = t1q[:, 0:42, 0:120].rearrange("p n (g c) -> p n g c", c=20)
                dstB = t1q[:, 42, 0:80].rearrange("p (g c) -> p g c", c=20)
                return {"t1q": t1q, "dstA": dstA, "dstB": dstB}

            def step1_b(st, psi_t, b):
                ps1 = ps_s1.tile([128, 4, 256], F32, tag="s1")  # 2 banks
                ps1_flat = ps1[:].rearrange("p c g -> p (c g)")
                for ac in range(8):
                    lhsT = lt_sb[:, b, ac]
                    psi_flat = psi_t[:, ac].rearrange("p c g -> p (c g)")
                    for cep in range(2):  # one 512-wide MM per PSUM bank
                        nc.tensor.matmul(
                            ps1_flat[:, cep * 512 : (cep + 1) * 512],
                            lhsT,
                            psi_flat[:, cep * 512 : (cep + 1) * 512],
                            start=(ac == 0),
                            stop=(ac == 7),
                        )
                nc.vector.tensor_copy(
                    st["dstA"][:, :, :, b * 4 : b * 4 + 2],
                    ps1[:, 0:2, 0:252].rearrange("p c (n g) -> p n g c", g=6),
                )
                nc.scalar.copy(
                    st["dstA"][:, :, :, b * 4 + 2 : b * 4 + 4],
                    ps1[:, 2:4, 0:252].rearrange("p c (n g) -> p n g c", g=6),
                )
                nc.vector.tensor_copy(
                    st["dstB"][:, :, b * 4 : b * 4 + 2],
                    ps1[:, 0:2, 252:256].rearrange("p c g -> p g c"),
                )
                nc.scalar.copy(
                    st["dstB"][:, :, b * 4 + 2 : b * 4 + 4],
                    ps1[:, 2:4, 252:256].rearrange("p c g -> p g c"),
                )

            # ---------- step 4: defer per-(ij,kh) groups into q+1's body
            def make_s4(qq, t3g_, rt_t):
                def emit(ij, kh, k0, k1):
                    ps4 = ps_s4.tile([128, 512], F32, tag="s4")  # 1 bank
                    kw = k1 - k0
                    for blk2 in range(2):
                        for f in range(5):
                            nc.tensor.matmul(
                                ps4[:, 0:kw],
                                t3g_[:, (ij * 5 + f) * 2 + blk2, :],
                                rt_t[:, blk2, f, kh * 512 + k0 : kh * 512 + k1],
                                start=(blk2 == 0 and f == 0),
                                stop=(blk2 == 1 and f == 4),
                            )
                    c0 = ij * 1024 + kh * 512 + k0
                    if qq == 0:
                        evac_copy(res_sb[:, c0 : c0 + kw], ps4[:, 0:kw])
                    elif qq < 3:
                        nc.vector.tensor_add(
                            res_sb[:, c0 : c0 + kw],
                            res_sb[:, c0 : c0 + kw],
                            ps4[:, 0:kw],
                        )
                    else:
                        # final quarter: add into the bf16 staging tile and
                        # ship it out on the scalar ring (idle at the tail)
                        nc.vector.tensor_add(
                            res_bf[:, c0 : c0 + kw],
                            res_sb[:, c0 : c0 + kw],
                            ps4[:, 0:kw],
                        )
                        nc.scalar.dma_start(
                            res.ap()[:, c0 : c0 + kw], res_bf[:, c0 : c0 + kw]
                        )

                groups = []
                for ij in range(4):
                    for kh in range(2):
                        if qq == 3 and ij == 3 and kh == 1:
                            # split the very last group so the final
                            # accumulate+DMA covers only 256 columns
                            groups.append(lambda ij=ij, kh=kh: emit(ij, kh, 0, 256))
                            groups.append(lambda ij=ij, kh=kh: emit(ij, kh, 256, 512))
                        else:
                            groups.append(lambda ij=ij, kh=kh: emit(ij, kh, 0, 512))
                return groups

            st_by_q = {}
            psis_by_q = {}
            rt_by_q = {}
            for q in range(4):  # g-quarters
                # ---------- step 1: T1q[h; pack, (g6, bce)] ----------
                if q == 0:
                    psis_by_q[0] = load_psi(0)
                    st_by_q[0] = alloc_t1q()
                    load_rest_of_consts()
                g0 = st_by_q[0]["t1q"][0:2, 0, 8:10] if q == 0 else None
                if q < 3:
                    psis_by_q[q + 1] = load_psi(q + 1, gate=g0)
                rt_by_q[q] = load_rt(q, gate=g0)
                # leading b-blocks of this quarter were hoisted into the
                # previous body (3 blocks into q0's flipA window, else 2)
                for b in range(0 if q == 0 else (3 if q == 1 else 2), 5):
                    step1_b(st_by_q[q], psis_by_q[q], b)
                    if b in ((3, 4) if q == 1 else (2, 4)):
                        fire_s4(1)  # step4(q-1) groups during step1(q)

                # ---------- flipA: DMA XBAR transpose to T1P[(g6,bce); pack, h]
                t1q_flat = st_by_q[q]["t1q"][:].rearrange("p n c -> p (n c)")
                t1p = t1p_pool.tile([128, NPACK, 128], BF16, tag="t1p")
                with tc.high_priority():
                    nc.scalar.dma_start(
                        t1p[:, 0:21, :], t1q_flat[:, 0 : 21 * 128], transpose=True
                    )
                    nc.sync.dma_start(
                        t1p[:, 21:NPACK, :], t1q_flat[:, 21 * 128 : NPACK * 128],
                        transpose=True,
                    )
                # hoist the next quarter's first two step-1 blocks here: they
                # keep the PE busy while the flipA XBAR transpose lands
                if q < 3:
                    st_by_q[q + 1] = alloc_t1q()
                    step1_b(st_by_q[q + 1], psis_by_q[q + 1], 0)
                    fire_s4(1)
                    step1_b(st_by_q[q + 1], psis_by_q[q + 1], 1)
                    fire_s4(1)
                    if q == 0:  # q0 has no deferred step-4 filler
                        step1_b(st_by_q[1], psis_by_q[1], 2)
                else:
                    fire_s4(2)
                fire_s4(1)

                # ---------- mix: T3H[h; ijf20, g256] ----------
                t3h = t3h_pool.tile([128, 20, 256], BF16, tag="t3h")
                groups = [list(range(g0_, min(g0_ + 4, NPACK))) for g0_ in range(0, NPACK, 4)]
                for gi, grp in enumerate(groups):
                    pmq = ps_mid.tile([128, 4, 128], F32, tag="mid")  # 1 bank
                    for k, pack in enumerate(grp):
                        if pack < 42:
                            nc.tensor.matmul(
                                pmq[:, k, :], t1p[0:120, pack, :], q6_sb[:],
                                start=True, stop=True,
                            )
                        else:
                            nc.tensor.matmul(
                                pmq[:, k, :], t1p[0:80, 42, :], q4_sb[:],
                                start=True, stop=True,
                            )
                    nfull = sum(1 for p_ in grp if p_ < 42)
                    if nfull:
                        evac_copy(
                            t3h[:, :, grp[0] * 6 : grp[0] * 6 + nfull * 6].rearrange(
                                "p i (k g) -> p k i g", g=6
                            ),
                            pmq[:, 0:nfull, 0:120].rearrange(
                                "p k (i g) -> p k i g", g=6
                            ),
                        )
                    if grp[-1] == 42:
                        evac_copy(
                            t3h[:, :, 252:256],
                            pmq[:, nfull, 0:80].rearrange("p (i g) -> p i g", g=4),
                        )
                    if gi in (2, 5):
                        fire_s4(1)
                fire_s4(1)

                # ---------- flipB: PE transpose to T3G[g; (ijf, blk), h]
                t3g = t3g_pool.tile([128, 40, 128], BF16, tag="t3g")
                if q == 3:
                    # last quarter: its step4 has no later phase to hide in,
                    # so interleave it into flipB as soon as data is ready
                    pending_s4.extend(make_s4(q, t3g, rt_by_q[q]))
                for ng in range(10):  # 4 transposes per PSUM bank
                    pb = ps_mid.tile([128, 4, 128], BF16, tag="mid")
                    for j in range(4):
                        n = ng * 4 + j  # n = ijf*2 + blk2
                        nc.tensor.transpose(
                            pb[:, j, :],
                            t3h[:, n // 2, (n % 2) * 128 : (n % 2) * 128 + 128],
                            idn_sb[:],
                        )
                    evac_copy(
                        t3g[:, ng * 4 : (ng + 1) * 4, :].rearrange("p n h -> p (n h)"),
                        pb[:].rearrange("p j h -> p (j h)"),
                    )
                    if q == 3 and ng in (3, 6):
                        fire_s4(2)  # ij0 after n<=15 done, ij1 after n<=27
                if q < 3:
                    pending_s4.extend(make_s4(q, t3g, rt_by_q[q]))

            # flush remaining deferred step-4 work (last quarter)
            fire_s4(len(pending_s4))
    nc.compile()
    return nc


def _host_inputs(psi_flat, L, M1, M2, R):
    # psi[a,ce,g] -> [ac, q, a_lo, ce, g256]
    psi = np.ascontiguousarray(
        psi_flat.reshape(8, 128, 4, 4, 256).transpose(0, 3, 1, 2, 4)
    ).astype(BF_NP)
    # R[f,k,g] -> RT[f,g,k] -> [blk, g_lo, f, k]
    RT = np.ascontiguousarray(
        R.transpose(2, 0, 1).reshape(8, 128, 5, 1024)
    ).astype(BF_NP)
    Q = np.einsum("bdic,dfje->bceijf", M1, M2).reshape(20, 20).astype(np.float32)
    rows = np.arange(20)
    Q6P = np.zeros((120, 128), np.float32)
    for g6 in range(6):
        Q6P[np.ix_(g6 * 20 + rows, rows * 6 + g6)] = Q
    Q4P = np.zeros((80, 128), np.float32)
    for g4 in range(4):
        Q4P[np.ix_(g4 * 20 + rows, rows * 4 + g4)] = Q
    Q6P = Q6P.astype(BF_NP)
    Q4P = Q4P.astype(BF_NP)
    idn = np.eye(128, dtype=np.float32).astype(BF_NP)
    in_maps = []
    for c in range(NCORES):
        LT = np.ascontiguousarray(
            L[:, c * H : (c + 1) * H, :].transpose(0, 2, 1).reshape(5, 8, 128, H)
            .transpose(0, 2, 1, 3)
        ).astype(BF_NP)  # [b, a_lo, ac, h]
        in_maps.append({"psi": psi, "lt": LT, "rt": RT, "q6": Q6P, "q4": Q4P, "idn": idn})
    return in_maps


def kernel(**inputs):
    psi_flat = np.asarray(inputs["psi_flat"], np.float32)
    L = np.asarray(inputs["L"], np.float32)
    M1 = np.asarray(inputs["M1"], np.float32)
    M2 = np.asarray(inputs["M2"], np.float32)
    R = np.asarray(inputs["R"], np.float32)

    global _nc_cache
    if _nc_cache is None:
        _nc_cache = _build_nc()
    nc = _nc_cache

    in_maps = _host_inputs(psi_flat, L, M1, M2, R)
    out = bass_utils.run_bass_kernel_spmd(nc, in_maps, core_ids=list(range(NCORES)))
    parts = [
        np.asarray(out.results[c]["res"]).astype(np.float32) for c in range(NCORES)
    ]
    return np.concatenate(parts, axis=0).reshape(-1)


# revision 37
# speedup vs baseline: 1.1443x; 1.0237x over previous
"""DMRG two-site effective Hamiltonian application (ApplyMPO) on 8 trn2 cores.

Math (reference):
  res[h,i,j,k] = sum_{a,b,c,d,e,f,g} L[b,h,a] M1[b,d,i,c] M2[d,f,j,e]
                                     R[f,k,g] psi[a,c,e,g]

Device algorithm (per core, output bond h sharded 8 x 128), all bf16 with
fp32 PSUM accumulation:
  Q[(b,c,e),(i,j,f)] = sum_d M1[b,d,i,c] M2[d,f,j,e]            (host, 400 els)
  step1: T1[h; pack, (g6,bce)] = sum_a L[b,h,a] psi[a,(c,e),g]  (PE, K=a)
         written in 128-col packs: col = g6*20 + (b*4+ce), 6 g's per pack
  flipA: T1P[(g6,bce); pack, h] = DMA XBAR transpose of T1       (DMA engine)
  mix:   T3H[h; (i,j,f), g]     = T1P[pack]^T @ Q6P              (PE, K=20 eff)
  flipB: T3G[g; (ijf,blk), h]   = PE transpose of T3H g-slices   (PE, 1 c/r)
  step4: res[h; ij,k]          += T3G[ijf,blk]^T @ R^T[f][g,k]   (PE, K=g, acc f)
flipA rides the DMA XBAR whose latency is hidden behind hoisted step-1 work;
flipB stays on the PE: moving it to the XBAR was tried and loses ~8us to
ring-FIFO serialization (flipA + prefetches + flipB all share the sync ring,
and two XBAR transposes in flight from different rings corrupt each other).

Scheduling notes (from trace analysis):
- A 10-matmul warmup accumulation group (one PSUM group -> no intermediate
  semaphores -> truly back-to-back) on a memset tile runs before any data
  lands so the PE HAM clock gate reaches 2.4 GHz (~3.4us of sustained
  activity) before the real matmuls; otherwise the first ~18us run at
  1.2 GHz.
- DMA trigger instructions cost ~600ns of sequencer time each, so psi/rt
  loads are batched (one trigger per quarter; q0's psi is split so the very
  first matmul gates on a 128KB slice, with the ac6-7 tail on the idle
  SWDGE ring for parallel transfer).
- psi(q1)/rt(q0) prefetches are gated on step1(q0)'s b2 output via a tiny
  SWDGE copy into their destination tiles, so the startup psi(q0) load gets
  the full HBM pipe (otherwise the prefetch transfers halve its bandwidth
  and the PE starves ~5us while HAM re-throttles).
- res accumulates fp32 in SBUF; the final (q3) add writes a bf16 staging
  tile DMAed out per 512-col chunk on the scalar ring (idle at the tail).
  The last group is split into two 256-col chunks to shorten the
  post-last-matmul tail.
"""

import numpy as np
import ml_dtypes

import concourse.bacc as bacc
import concourse.mybir as mybir
import concourse.tile as tile
from concourse import bass_utils

F32 = mybir.dt.float32
BF16 = mybir.dt.bfloat16
BF_NP = ml_dtypes.bfloat16

CHI = 1024
W = 5
D = 2
NCORES = 8
H = CHI // NCORES  # 128, h rows per core
NPACK = 43  # 6-g packs per 256-g quarter: 42 full + one 4-g tail

_nc_cache = None


def _build_nc():
    nc = bacc.Bacc("TRN2", target_bir_lowering=False)
    # host-prearranged: psi[ac, q, a_lo, ce, g256]; lt[b, a_lo, ac, h]; rt[blk, g_lo, f, k]
    psi = nc.dram_tensor("psi", [8, 4, 128, 4, 256], BF16, kind="ExternalInput")
    lt = nc.dram_tensor("lt", [5, 128, 8, H], BF16, kind="ExternalInput")
    rt = nc.dram_tensor("rt", [8, 128, 5, 1024], BF16, kind="ExternalInput")
    q6 = nc.dram_tensor("q6", [120, 128], BF16, kind="ExternalInput")
    q4 = nc.dram_tensor("q4", [80, 128], BF16, kind="ExternalInput")
    idn = nc.dram_tensor("idn", [128, 128], BF16, kind="ExternalInput")
    res = nc.dram_tensor("res", [H, 4096], BF16, kind="ExternalOutput")  # h;(i,j,k)

    with tile.TileContext(nc) as tc:
        with (
            tc.tile_pool(name="const", bufs=1) as const_pool,
            tc.tile_pool(name="psis", bufs=2) as psi_pool,
            tc.tile_pool(name="t1", bufs=2) as t1_pool,
            tc.tile_pool(name="t1p", bufs=2) as t1p_pool,
            tc.tile_pool(name="t3h", bufs=2) as t3h_pool,
            tc.tile_pool(name="t3g", bufs=2) as t3g_pool,
            tc.tile_pool(name="rt_p", bufs=2) as rt_pool,
            tc.tile_pool(name="resp", bufs=1) as res_pool,
            tc.tile_pool(name="ps_s1", bufs=2, space="PSUM") as ps_s1,
            tc.tile_pool(name="ps_mid", bufs=2, space="PSUM") as ps_mid,
            tc.tile_pool(name="ps_s4", bufs=2, space="PSUM") as ps_s4,
        ):
            # ---- PE warmup ----
            warm_src = const_pool.tile([128, 512], BF16)
            nc.vector.memset(warm_src[:], 1.0)
            wps = ps_s4.tile([128, 512], F32, tag="s4")
            NWARM = 10
            for i in range(NWARM):
                nc.tensor.matmul(
                    wps[:], warm_src[:, 0:128], warm_src[:],
                    start=(i == 0), stop=(i == NWARM - 1),
                )

            # ---- static loads (only b=0 weights gate the first matmul) ----
            lt_sb = const_pool.tile([128, 5, 8, H], BF16)  # [a_lo; b, ac, h]
            lt_r = lt.ap().rearrange("b p ac h -> p b ac h")
            nc.scalar.dma_start(lt_sb[:, 0, 0], lt_r[:, 0, 0])
            nc.scalar.dma_start(lt_sb[:, 0, 1:8], lt_r[:, 0, 1:8])
            q6_sb = const_pool.tile([120, 128], BF16)
            q4_sb = const_pool.tile([80, 128], BF16)
            idn_sb = const_pool.tile([128, 128], BF16)
            res_sb = res_pool.tile([128, 4096], F32)
            res_bf = res_pool.tile([128, 4096], BF16)

            def load_rest_of_consts():
                for b in range(1, 5):
                    nc.scalar.dma_start(lt_sb[:, b], lt_r[:, b])
                nc.scalar.dma_start(q6_sb[:], q6.ap())
                nc.scalar.dma_start(q4_sb[:], q4.ap())
                nc.scalar.dma_start(idn_sb[:], idn.ap())

            evac_ct = 0

            def evac_copy(out, in_):
                # DVE is ~2x faster than ACT for copies; give ACT 1 in 3.
                nonlocal evac_ct
                evac_ct += 1
                if evac_ct % 3 == 0:
                    nc.scalar.copy(out, in_)
                else:
                    nc.vector.tensor_copy(out, in_)

            # deferred step-4 work: list of closures (one per psum group)
            pending_s4 = []

            def fire_s4(n=1):
                for _ in range(n):
                    if pending_s4:
                        pending_s4.pop(0)()

            psi_r = psi.ap()

            def load_psi(qq, gate=None):
                # one SBUF tile per quarter: [a_lo; ac, ce, g256]
                pt = psi_pool.tile([128, 8, 4, 256], BF16, tag="psi")
                if gate is not None:
                    # tiny copy from the gate region into the DMA dest: the
                    # transfer then cannot start before the gate is written
                    # (startup bandwidth protection)
                    nc.gpsimd.tensor_copy(pt[0:2, 0, 0, 0:2], gate)
                if qq == 0:
                    nc.sync.dma_start(pt[:, 0, 0:2], psi_r[0, 0, :, 0:2])
                    nc.sync.dma_start(pt[:, 0, 2:4], psi_r[0, 0, :, 2:4])
                    nc.sync.dma_start(
                        pt[:, 1:3], psi_r[1:3, 0].rearrange("ac p ce g -> p ac ce g")
                    )
                    nc.sync.dma_start(
                        pt[:, 3:6], psi_r[3:6, 0].rearrange("ac p ce g -> p ac ce g")
                    )
                    nc.gpsimd.dma_start(
                        pt[:, 6:8], psi_r[6:8, 0].rearrange("ac p ce g -> p ac ce g")
                    )
                elif gate is not None:
                    # q1 prefetch: one trigger, dispatch held back by the
                    # gate until step1(q0) is past b2
                    nc.sync.dma_start(
                        pt[:], psi_r[:, qq].rearrange("ac p ce g -> p ac ce g")
                    )
                else:
                    # steady state: fine-grained triggers so the scheduler
                    # can slot the urgent flipA XBAR between them on the
                    # ring (a single 2.1MB transfer blocks it for ~6us)
                    for ac in range(8):
                        nc.sync.dma_start(pt[:, ac], psi_r[ac, qq])
                return pt

            def load_rt(qq, gate=None):
                rt_t = rt_pool.tile([128, 2, 5, 1024], BF16, tag="rt")
                if gate is not None:
                    # startup: one gated trigger (bandwidth protection)
                    nc.gpsimd.tensor_copy(rt_t[0:2, 0, 0, 0:2], gate)
                    nc.sync.dma_start(
                        rt_t[:],
                        rt.ap()[qq * 2 : qq * 2 + 2].rearrange(
                            "blk p f k -> p blk f k"
                        ),
                    )
                else:
                    for blk2 in range(2):
                        nc.sync.dma_start(rt_t[:, blk2], rt.ap()[qq * 2 + blk2])
                return rt_t

            def alloc_t1q():
                t1q = t1_pool.tile([128, NPACK, 128], BF16, tag="t1q")
                # packed destination views: col = g6*20 + (b*4 + ce)
                dstA = t1q[:, 0:42, 0:120].rearrange("p n (g c) -> p n g c", c=20)
                dstB = t1q[:, 42, 0:80].rearrange("p (g c) -> p g c", c=20)
                return {"t1q": t1q, "dstA": dstA, "dstB": dstB}

            def step1_b(st, psi_t, b):
                ps1 = ps_s1.tile([128, 4, 256], F32, tag="s1")  # 2 banks
                ps1_flat = ps1[:].rearrange("p c g -> p (c g)")
                for ac in range(8):
                    lhsT = lt_sb[:, b, ac]
                    psi_flat = psi_t[:, ac].rearrange("p c g -> p (c g)")
                    for cep in range(2):  # one 512-wide MM per PSUM bank
                        nc.tensor.matmul(
                            ps1_flat[:, cep * 512 : (cep + 1) * 512],
                            lhsT,
                            psi_flat[:, cep * 512 : (cep + 1) * 512],
                            start=(ac == 0),
                            stop=(ac == 7),
                        )
                nc.vector.tensor_copy(
                    st["dstA"][:, :, :, b * 4 : b * 4 + 2],
                    ps1[:, 0:2, 0:252].rearrange("p c (n g) -> p n g c", g=6),
                )
                nc.scalar.copy(
                    st["dstA"][:, :, :, b * 4 + 2 : b * 4 + 4],
                    ps1[:, 2:4, 0:252].rearrange("p c (n g) -> p n g c", g=6),
                )
                nc.vector.tensor_copy(
                    st["dstB"][:, :, b * 4 : b * 4 + 2],
                    ps1[:, 0:2, 252:256].rearrange("p c g -> p g c"),
                )
                nc.scalar.copy(
                    st["dstB"][:, :, b * 4 + 2 : b * 4 + 4],
                    ps1[:, 2:4, 252:256].rearrange("p c g -> p g c"),
                )

            # ---------- step 4: defer per-(ij,kh) groups into q+1's body
            def make_s4(qq, t3g_, rt_t):
                def emit(ij, kh, k0, k1):
                    ps4 = ps_s4.tile([128, 512], F32, tag="s4")  # 1 bank
                    kw = k1 - k0
                    for blk2 in range(2):
                        for f in range(5):
                            nc.tensor.matmul(
                                ps4[:, 0:kw],
                                t3g_[:, (ij * 5 + f) * 2 + blk2, :],
                                rt_t[:, blk2, f, kh * 512 + k0 : kh * 512 + k1],
                                start=(blk2 == 0 and f == 0),
                                stop=(blk2 == 1 and f == 4),
                            )
                    c0 = ij * 1024 + kh * 512 + k0
                    if qq == 0:
                        evac_copy(res_sb[:, c0 : c0 + kw], ps4[:, 0:kw])
                    elif qq < 3:
                        nc.vector.tensor_add(
                            res_sb[:, c0 : c0 + kw],
                            res_sb[:, c0 : c0 + kw],
                            ps4[:, 0:kw],
                        )
                    else:
                        # final quarter: add into the bf16 staging tile and
                        # ship it out on the scalar ring (idle at the tail)
                        nc.vector.tensor_add(
                            res_bf[:, c0 : c0 + kw],
                            res_sb[:, c0 : c0 + kw],
                            ps4[:, 0:kw],
                        )
                        nc.scalar.dma_start(
                            res.ap()[:, c0 : c0 + kw], res_bf[:, c0 : c0 + kw]
                        )

                groups = []
                for ij in range(4):
                    for kh in range(2):
                        if qq == 3 and ij == 3 and kh == 1:
                            # split the very last group so the final
                            # accumulate+DMA covers only 256 columns
                            groups.append(lambda ij=ij, kh=kh: emit(ij, kh, 0, 256))
                            groups.append(lambda ij=ij, kh=kh: emit(ij, kh, 256, 512))
                        else:
                            groups.append(lambda ij=ij, kh=kh: emit(ij, kh, 0, 512))
                return groups

            st_by_q = {}
            psis_by_q = {}
            rt_by_q = {}
            for q in range(4):  # g-quarters
                # ---------- step 1: T1q[h; pack, (g6, bce)] ----------
                if q == 0:
                    psis_by_q[0] = load_psi(0)
                    st_by_q[0] = alloc_t1q()
                    load_rest_of_consts()
                g0 = st_by_q[0]["t1q"][0:2, 0, 8:10] if q == 0 else None
                if q < 3:
                    psis_by_q[q + 1] = load_psi(q + 1, gate=g0)
                rt_by_q[q] = load_rt(q, gate=g0)
                # leading b-blocks of this quarter were hoisted into the
                # previous body (3 blocks into q0's flipA window, else 2)
                for b in range(0 if q == 0 else (3 if q == 1 else 2), 5):
                    step1_b(st_by_q[q], psis_by_q[q], b)
                    if b in ((3, 4) if q == 1 else (2, 4)):
                        fire_s4(1)  # step4(q-1) groups during step1(q)

                # ---------- flipA: DMA XBAR transpose to T1P[(g6,bce); pack, h]
                t1q_flat = st_by_q[q]["t1q"][:].rearrange("p n c -> p (n c)")
                t1p = t1p_pool.tile([128, NPACK, 128], BF16, tag="t1p")
                with tc.high_priority():
                    nc.scalar.dma_start(
                        t1p[:, 0:21, :], t1q_flat[:, 0 : 21 * 128], transpose=True
                    )
                    nc.sync.dma_start(
                        t1p[:, 21:NPACK, :], t1q_flat[:, 21 * 128 : NPACK * 128],
                        transpose=True,
                    )
                # hoist the next quarter's first two step-1 blocks here: they
                # keep the PE busy while the flipA XBAR transpose lands
                if q < 3:
                    st_by_q[q + 1] = alloc_t1q()
                    step1_b(st_by_q[q + 1], psis_by_q[q + 1], 0)
                    fire_s4(1)
                    step1_b(st_by_q[q + 1], psis_by_q[q + 1], 1)
                    fire_s4(1)
                    if q == 0:  # q0 has no deferred step-4 filler
                        step1_b(st_by_q[1], psis_by_q[1], 2)
                else:
                    fire_s4(2)
                fire_s4(1)

                # ---------- mix: T3H[h; ijf20, g256] ----------
                t3h = t3h_pool.tile([128, 20, 256], BF16, tag="t3h")
                groups = [list(range(g0_, min(g0_ + 4, NPACK))) for g0_ in range(0, NPACK, 4)]
                for gi, grp in enumerate(groups):
                    pmq = ps_mid.tile([128, 4, 128], F32, tag="mid")  # 1 bank
                    for k, pack in enumerate(grp):
                        if pack < 42:
                            nc.tensor.matmul(
                                pmq[:, k, :], t1p[0:120, pack, :], q6_sb[:],
                                start=True, stop=True,
                            )
                        else:
                            nc.tensor.matmul(
                                pmq[:, k, :], t1p[0:80, 42, :], q4_sb[:],
                                start=True, stop=True,
                            )
                    nfull = sum(1 for p_ in grp if p_ < 42)
                    if nfull:
                        evac_copy(
                            t3h[:, :, grp[0] * 6 : grp[0] * 6 + nfull * 6].rearrange(
                                "p i (k g) -> p k i g", g=6
                            ),
                            pmq[:, 0:nfull, 0:120].rearrange(
                                "p k (i g) -> p k i g", g=6
                            ),
                        )
                    if grp[-1] == 42:
                        evac_copy(
                            t3h[:, :, 252:256],
                            pmq[:, nfull, 0:80].rearrange("p (i g) -> p i g", g=4),
                        )
                    if gi in (2, 5):
                        fire_s4(1)
                fire_s4(1)

                # ---------- flipB: PE transpose to T3G[g; (ijf, blk), h]
                t3g = t3g_pool.tile([128, 40, 128], BF16, tag="t3g")
                if q == 3:
                    # last quarter: its step4 has no later phase to hide in,
                    # so interleave it into flipB as soon as data is ready
                    pending_s4.extend(make_s4(q, t3g, rt_by_q[q]))
                for ng in range(10):  # 4 transposes per PSUM bank
                    pb = ps_mid.tile([128, 4, 128], BF16, tag="mid")
                    for j in range(4):
                        n = ng * 4 + j  # n = ijf*2 + blk2
                        nc.tensor.transpose(
                            pb[:, j, :],
                            t3h[:, n // 2, (n % 2) * 128 : (n % 2) * 128 + 128],
                            idn_sb[:],
                        )
                    evac_copy(
                        t3g[:, ng * 4 : (ng + 1) * 4, :].rearrange("p n h -> p (n h)"),
                        pb[:].rearrange("p j h -> p (j h)"),
                    )
                    if q == 3 and ng in (3, 6):
                        fire_s4(2)  # ij0 after n<=15 done, ij1 after n<=27
                if q < 3:
                    pending_s4.extend(make_s4(q, t3g, rt_by_q[q]))

            # flush remaining deferred step-4 work (last quarter)
            fire_s4(len(pending_s4))
    nc.compile()
    return nc


def _host_inputs(psi_flat, L, M1, M2, R):
    # psi[a,ce,g] -> [ac, q, a_lo, ce, g256]
    psi = np.ascontiguousarray(
        psi_flat.reshape(8, 128, 4, 4, 256).transpose(0, 3, 1, 2, 4)
    ).astype(BF_NP)
    # R[f,k,g] -> RT[f,g,k] -> [blk, g_lo, f, k]
    RT = np.ascontiguousarray(
        R.transpose(2, 0, 1).reshape(8, 128, 5, 1024)
    ).astype(BF_NP)
    Q = np.einsum("bdic,dfje->bceijf", M1, M2).reshape(20, 20).astype(np.float32)
    rows = np.arange(20)
    Q6P = np.zeros((120, 128), np.float32)
    for g6 in range(6):
        Q6P[np.ix_(g6 * 20 + rows, rows * 6 + g6)] = Q
    Q4P = np.zeros((80, 128), np.float32)
    for g4 in range(4):
        Q4P[np.ix_(g4 * 20 + rows, rows * 4 + g4)] = Q
    Q6P = Q6P.astype(BF_NP)
    Q4P = Q4P.astype(BF_NP)
    idn = np.eye(128, dtype=np.float32).astype(BF_NP)
    in_maps = []
    for c in range(NCORES):
        LT = np.ascontiguousarray(
            L[:, c * H : (c + 1) * H, :].transpose(0, 2, 1).reshape(5, 8, 128, H)
            .transpose(0, 2, 1, 3)
        ).astype(BF_NP)  # [b, a_lo, ac, h]
        in_maps.append({"psi": psi, "lt": LT, "rt": RT, "q6": Q6P, "q4": Q4P, "idn": idn})
    return in_maps


def kernel(**inputs):
    psi_flat = np.asarray(inputs["psi_flat"], np.float32)
    L = np.asarray(inputs["L"], np.float32)
    M1 = np.asarray(inputs["M1"], np.float32)
    M2 = np.asarray(inputs["M2"], np.float32)
    R = np.asarray(inputs["R"], np.float32)

    global _nc_cache
    if _nc_cache is None:
        _nc_cache = _build_nc()
    nc = _nc_cache

    in_maps = _host_inputs(psi_flat, L, M1, M2, R)
    out = bass_utils.run_bass_kernel_spmd(nc, in_maps, core_ids=list(range(NCORES)))
    parts = [
        np.asarray(out.results[c]["res"]).astype(np.float32) for c in range(NCORES)
    ]
    return np.concatenate(parts, axis=0).reshape(-1)
